# revision 24
# baseline (speedup 1.0000x reference)
"""Trainium2 Bass kernel for nn_Attention_3736621547687.

B=1, S=2048, HID=2048, NH=16, NKV=4, HD=128 attention block:
qkv proj -> per-head RMSNorm(q,k) -> RoPE -> causal GQA attention -> o proj.

Sharding: tensor-parallel over heads across 8 cores. Core c owns q heads
{2c, 2c+1} and kv head c//2 (replicated across the pair of cores sharing it).
Each core computes a partial o-projection output; the host sums the 8
partials (Megatron-style row-parallel reduce) and adds the output bias.

Device-side layout trick: everything is computed in "transposed" orientation
(feature dim on partitions, sequence on the free dim) so that no on-chip
transposes of activations are needed:
  - host supplies hidden^T, wqkv_c^T, wo_c^T, cos/sin tiled to [128, S]
  - qkv proj emits q^T/k^T/v^T directly
  - scores are computed as scores^T [keys, queries]; softmax denominators are
    partition-dim sums obtained with an all-ones [128,128] matmul that also
    replicates the result across partitions (giving the broadcast for free)
  - exp() is fused with the 1/sqrt(qpa) scale on the scalar engine; causal
    masking = multiplying exp values by a 0/1 band mask on the vector engine
    (identical to the reference's additive -1e9 mask in fp32)
  - softmax max-subtraction is skipped: scores are ~N(0,1) after RMSNorm so
    exp() cannot overflow; mathematically identical to the reference.
All activations/weights are bf16 (fast weight load, 2x DVE modes, half the
HBM traffic); accumulations stay in fp32 PSUM.
"""

import numpy as np
from contextlib import ExitStack

import concourse.bass as bass
import concourse.bacc as bacc
import concourse.mybir as mybir
import concourse.tile as tile
from concourse.masks import make_identity
from concourse.bass_utils import run_bass_kernel_spmd

S = 2048
HID = 2048
NH = 16
NKV = 4
HD = 128
G = NH // NKV
SCALE = float(128.0 ** -0.5)  # query_pre_attn_scalar = 128
EPS = 1e-6

FP32 = mybir.dt.float32
F32R = mybir.dt.float32r
BF16 = mybir.dt.bfloat16
MULT = mybir.AluOpType.mult
AF = mybir.ActivationFunctionType

N_CORES = 8
SC_ATT = 512    # attention/oproj moving-dim chunk


def _patch_act_tables():
    """Force Ln and Exp onto the single combined activation-table set so the
    scalar engine never reloads tables when rms-norm and softmax interleave.
    Set ids must keep their positions, so competing sets are emptied rather
    than removed."""
    import concourse.hw_specs as hw_specs
    import concourse.bacc as bacc_mod
    orig = hw_specs.get_activation_tables

    def patched(module_arch):
        t = orig(module_arch)
        for name in ("exp_and_others", "natural_log", "exp_and_friends"):
            if name in t and "natural_log_exp_and_others" in t:
                t[name] = set()
        return t

    bacc_mod.get_activation_tables = patched


def build_nc():
    _patch_act_tables()
    nc = bacc.Bacc()

    NCH = S // SC_ATT
    hT = nc.dram_tensor("hT", [128, NCH, 16, SC_ATT], BF16,
                        kind="ExternalInput")
    wts_d = [nc.dram_tensor(f"wT{i}", [128, 16, HD], BF16, kind="ExternalInput")
             for i in range(4)]
    b4 = nc.dram_tensor("b4", [128, 4], FP32, kind="ExternalInput")
    woT = nc.dram_tensor("woT", [128, 2, HID], BF16, kind="ExternalInput")
    # rope tables with the rms-norm weight folded in per partition:
    # qc2 = qw*[cos;cos], qs2 = swap(qw)*[sin;sin], likewise for k
    qc2 = nc.dram_tensor("qc2", [128, S], BF16, kind="ExternalInput")
    qs2 = nc.dram_tensor("qs2", [128, S], BF16, kind="ExternalInput")
    kc2 = nc.dram_tensor("kc2", [128, S], BF16, kind="ExternalInput")
    ks2 = nc.dram_tensor("ks2", [128, S], BF16, kind="ExternalInput")
    onesd = nc.dram_tensor("ones", [128, 128], F32R, kind="ExternalInput")
    onesbd = nc.dram_tensor("onesb", [128, 128], BF16, kind="ExternalInput")
    rswapd = nc.dram_tensor("rswap", [128, 128], BF16, kind="ExternalInput")
    maskd = nc.dram_tensor("mk", [128, 4, SC_ATT], BF16, kind="ExternalInput")
    outp = nc.dram_tensor("outp", [S, HID], BF16, kind="ExternalOutput")

    with ExitStack() as ctx:
        tc = ctx.enter_context(tile.TileContext(nc))

        const = ctx.enter_context(tc.tile_pool(name="const", bufs=1))
        hpool = ctx.enter_context(tc.tile_pool(name="hpool", bufs=2))
        rawp = ctx.enter_context(tc.tile_pool(name="rawp", bufs=1))
        atp = ctx.enter_context(tc.tile_pool(name="atp", bufs=1))
        wpool = ctx.enter_context(tc.tile_pool(name="wpool", bufs=3))
        vpool = ctx.enter_context(tc.tile_pool(name="vpool", bufs=1))
        ppool = ctx.enter_context(tc.tile_pool(name="ppool", bufs=2))
        dpool = ctx.enter_context(tc.tile_pool(name="dpool", bufs=2))
        opool = ctx.enter_context(tc.tile_pool(name="opool", bufs=2))

        psA = ctx.enter_context(tc.tile_pool(name="psA", bufs=2, space="PSUM"))
        psB = ctx.enter_context(tc.tile_pool(name="psB", bufs=2, space="PSUM"))
        psC = ctx.enter_context(tc.tile_pool(name="psC", bufs=2, space="PSUM"))
        psN = ctx.enter_context(tc.tile_pool(name="psN", bufs=1, space="PSUM"))
        psO = ctx.enter_context(tc.tile_pool(name="psO", bufs=1, space="PSUM"))

        # ---- startup-critical loads: wts2 leads the sync ring; the
        # chunk-0 htsA tile rides the SWDGE ring (no activation-table
        # load ahead of it), so the first matmul starts ~4us in
        wts = [None] * 4
        wts[2] = const.tile([128, 16, HD], BF16, name="wts2", tag="wts2")
        nc.sync.dma_start(out=wts[2], in_=wts_d[2][:, :, :])
        hts0A = hpool.tile([128, 8, SC_ATT], BF16, tag="htsA")
        nc.gpsimd.dma_start(out=hts0A, in_=hT[:, 0, 0:8, :])

        # ---- small constants on the SWDGE queue ------------------------
        ident = const.tile([128, 128], BF16)
        make_identity(nc, ident)
        ones128 = const.tile([128, 128], F32R)
        nc.gpsimd.dma_start(out=ones128, in_=onesd[:, :])
        onesb = const.tile([128, 128], BF16)
        nc.gpsimd.dma_start(out=onesb, in_=onesbd[:, :])
        Rm = const.tile([128, 128], BF16)
        nc.gpsimd.dma_start(out=Rm, in_=rswapd[:, :])
        b4s = const.tile([128, 4], FP32)
        nc.gpsimd.dma_start(out=b4s, in_=b4[:, :])
        kc2s = const.tile([128, S], BF16)
        nc.gpsimd.dma_start(out=kc2s, in_=kc2[:, :])
        ks2s = const.tile([128, S], BF16)
        nc.gpsimd.dma_start(out=ks2s, in_=ks2[:, :])
        qc2s = const.tile([128, S], BF16)
        nc.gpsimd.dma_start(out=qc2s, in_=qc2[:, :])
        qs2s = const.tile([128, S], BF16)
        nc.gpsimd.dma_start(out=qs2s, in_=qs2[:, :])
        epsc = const.tile([128, 1], FP32)
        nc.vector.memset(epsc, EPS)
        mks = const.tile([128, 4, SC_ATT], BF16)
        nc.gpsimd.dma_start(out=mks, in_=maskd[:, :, :])

        rawq = [rawp.tile([128, S], BF16, tag=f"raw{i}", name=f"raw{i}")
                for i in range(3)]
        qhat = [atp.tile([128, S], BF16, tag=f"qh{i}", name=f"qh{i}")
                for i in range(2)]
        khat = atp.tile([128, S], BF16, tag="kh")
        attnT = [atp.tile([128, S], BF16, tag=f"attnT{h}", name=f"attnT{h}")
                 for h in range(2)]
        vsb = vpool.tile([128, 16, HD], BF16, tag="vsb")
        heads = [
            (rawq[2], khat, kc2s, ks2s),
            (rawq[0], qhat[0], qc2s, qs2s),
            (rawq[1], qhat[1], qc2s, qs2s),
        ]
        raw3s = {}

        # ================================================================
        # Software-pipelined emission: stage lag guarantees every
        # instruction's inputs are a full pipeline iteration old, so no
        # engine stream ever blocks at a phase boundary.
        #   iter sc: proj(sc) | rope+V(sc-1) | attention(sc-2) | oproj(sc-3)
        # ================================================================
        for it in range(NCH + 3):
            # ---- stage 1: qkv projection ------------------------------
            if it < NCH:
                sc = it
                sl = bass.ts(sc, SC_ATT)
                if sc == 0:
                    htsA = hts0A  # preloaded at the head of the SWDGE ring
                else:
                    htsA = hpool.tile([128, 8, SC_ATT], BF16, tag="htsA")
                    nc.sync.dma_start(out=htsA, in_=hT[:, sc, 0:8, :])
                htsB = hpool.tile([128, 8, SC_ATT], BF16, tag="htsB")
                nc.scalar.dma_start(out=htsB, in_=hT[:, sc, 8:16, :])
                if sc == 0:
                    # remaining weights follow wts2 on the sync ring;
                    # woT rides the SWDGE queue (needed latest)
                    for oc in (0, 1, 3):
                        wt = const.tile([128, 16, HD], BF16,
                                        name=f"wts{oc}", tag=f"wts{oc}")
                        nc.sync.dma_start(out=wt, in_=wts_d[oc][:, :, :])
                        wts[oc] = wt
                    woTs = const.tile([128, 2, HID], BF16)
                    nc.gpsimd.dma_start(out=woTs, in_=woT[:, :, :])
                for oc in (2, 0, 1, 3):
                    ps = psA.tile([128, SC_ATT], FP32, tag="mm")
                    for kt in range(16):
                        src_h = htsA if kt < 8 else htsB
                        nc.tensor.matmul(
                            ps, lhsT=wts[oc][:, kt, :],
                            rhs=src_h[:, kt % 8, :],
                            start=(kt == 0), stop=(kt == 15))
                    if oc == 3:
                        raw3 = wpool.tile([128, SC_ATT], BF16, tag="raw3",
                                          bufs=2)
                        nc.vector.tensor_scalar_add(raw3, ps,
                                                    b4s[:, oc:oc + 1])
                        raw3s[sc] = raw3
                    else:
                        nc.vector.tensor_scalar_add(
                            rawq[oc][:, sl], ps, b4s[:, oc:oc + 1])

            # ---- stage 2: rmsnorm + rope + V tiles --------------------
            if 1 <= it <= NCH:
                sc = it - 1
                sl = bass.ts(sc, SC_ATT)
                for raw, dst, wc2_, ws2_ in heads:
                    sq = wpool.tile([128, SC_ATT], BF16, tag="sq", bufs=2)
                    nc.vector.tensor_mul(sq, raw[:, sl], raw[:, sl])
                    ssum = psN.tile([128, SC_ATT], FP32, tag="norm")
                    nc.tensor.matmul(ssum, lhsT=onesb, rhs=sq,
                                     start=True, stop=True)
                    lnb = wpool.tile([128, SC_ATT], BF16, tag="lnb", bufs=2)
                    nc.scalar.activation(lnb, ssum, AF.Ln,
                                         scale=1.0 / HD, bias=epsc)
                    nc.scalar.activation(lnb, lnb, AF.Exp, scale=-0.5)
                    rtp = psN.tile([128, SC_ATT], FP32, tag="norm")
                    nc.tensor.matmul(rtp, lhsT=Rm, rhs=raw[:, sl],
                                     start=True, stop=True)
                    t1 = wpool.tile([128, SC_ATT], BF16, tag="tt", bufs=3)
                    nc.vector.tensor_mul(t1, raw[:, sl], wc2_[:, sl])
                    t2 = wpool.tile([128, SC_ATT], BF16, tag="tt", bufs=3)
                    nc.vector.tensor_mul(t2, rtp, ws2_[:, sl])
                    t3 = wpool.tile([128, SC_ATT], BF16, tag="tt", bufs=3)
                    nc.vector.tensor_add(t3, t1, t2)
                    nc.vector.tensor_mul(dst[:, sl], t3, lnb)
                raw3 = raw3s.pop(sc)
                for j in range(4):
                    tt = 4 * sc + j
                    vps = psN.tile([128, 128], BF16, tag="norm")
                    nc.tensor.transpose(vps,
                                        raw3[:, bass.ts(j, 128)], ident)
                    nc.vector.tensor_copy(vsb[:, tt, :], vps)

            # ---- stage 3: attention, both heads -----------------------
            # The last query chunk is processed in two 256-wide halves:
            # the first half skips the top key tiles entirely (less exp
            # work) and its o-projection overlaps the second half's
            # attention, shortening the serial tail of the kernel.
            if 2 <= it <= NCH + 1:
                sc = it - 2
                if sc < NCH - 1:
                    subs = [(sc * SC_ATT, SC_ATT)]
                else:
                    subs = [(sc * SC_ATT, SC_ATT // 2),
                            (sc * SC_ATT + SC_ATT // 2, SC_ATT // 2)]
                for q0, qn in subs:
                    for h in range(2):
                        ntt = (q0 + qn) // 128
                        outps = psO.tile([128, qn], FP32, tag="attnout")
                        dacc = dpool.tile([128, qn], F32R, tag="dacc")
                        t0 = 0
                        paired = qn * 2 * 4 <= 2048  # 2 tiles fit one bank
                        while t0 < ntt:
                            gn = min(4, ntt - t0)
                            pg = ppool.tile([128, 4, qn], BF16, tag="pt")
                            if paired:
                                # two key tiles share one PSUM bank and a
                                # single exp ACTIVATE (halves ACT op count)
                                for jp in range(0, gn, 2):
                                    scp2 = psC.tile([128, 2, qn], FP32,
                                                    tag="score")
                                    for u in range(2):
                                        tt = t0 + jp + u
                                        nc.tensor.matmul(
                                            scp2[:, u, :],
                                            lhsT=khat[:, bass.ts(tt, 128)],
                                            rhs=qhat[h][:, bass.ds(q0, qn)],
                                            start=True, stop=True)
                                    nc.scalar.activation(
                                        pg[:, jp:jp + 2, :], scp2,
                                        AF.Exp, scale=SCALE)
                                    for u in range(2):
                                        tt = t0 + jp + u
                                        j = jp + u
                                        jd = tt - q0 // 128
                                        if jd >= 0:
                                            nc.vector.tensor_mul(
                                                pg[:, j, :], pg[:, j, :],
                                                mks[:, jd, 0:qn])
                                        nc.tensor.matmul(
                                            outps, lhsT=vsb[:, tt, :],
                                            rhs=pg[:, j, :],
                                            start=(tt == 0),
                                            stop=(tt == ntt - 1))
                            else:
                                for j in range(gn):
                                    tt = t0 + j
                                    scp = psC.tile([128, qn], FP32,
                                                   tag="score")
                                    nc.tensor.matmul(
                                        scp, lhsT=khat[:, bass.ts(tt, 128)],
                                        rhs=qhat[h][:, bass.ds(q0, qn)],
                                        start=True, stop=True)
                                    nc.scalar.activation(pg[:, j, :], scp,
                                                         AF.Exp,
                                                         scale=SCALE)
                                    jd = tt - q0 // 128
                                    if jd >= 0:  # diagonal: zero t > s
                                        nc.vector.tensor_mul(
                                            pg[:, j, :], pg[:, j, :],
                                            mks[:, jd, 0:qn])
                                    nc.tensor.matmul(outps,
                                                     lhsT=vsb[:, tt, :],
                                                     rhs=pg[:, j, :],
                                                     start=(tt == 0),
                                                     stop=(tt == ntt - 1))
                            if gn == 4:
                                ga = ppool.tile([128, qn], BF16, tag="ga",
                                                bufs=2)
                                nc.vector.tensor_add(ga, pg[:, 0, :],
                                                     pg[:, 1, :])
                                gb = ppool.tile([128, qn], BF16, tag="gb",
                                                bufs=1)
                                nc.vector.tensor_add(gb, pg[:, 2, :],
                                                     pg[:, 3, :])
                                if t0 == 0:
                                    nc.vector.tensor_add(dacc, ga, gb)
                                else:
                                    gc = ppool.tile([128, qn], BF16,
                                                    tag="ga", bufs=2)
                                    nc.vector.tensor_add(gc, ga, gb)
                                    nc.vector.tensor_add(dacc, dacc, gc)
                            else:  # trailing pair (only in split chunks)
                                ga = ppool.tile([128, qn], BF16, tag="ga",
                                                bufs=2)
                                nc.vector.tensor_add(ga, pg[:, 0, :],
                                                     pg[:, 1, :])
                                nc.vector.tensor_add(dacc, dacc, ga)
                            t0 += gn
                        drep = psN.tile([128, qn], FP32, tag="norm")
                        nc.tensor.matmul(drep, lhsT=ones128, rhs=dacc,
                                         start=True, stop=True)
                        drec = wpool.tile([128, qn], FP32, tag="drec",
                                          bufs=2)
                        nc.vector.reciprocal_approx_fast(drec, drep)
                        nc.vector.tensor_mul(attnT[h][:, bass.ds(q0, qn)],
                                             outps, drec)

            # ---- stage 4: o projection --------------------------------
            if it >= 3:
                sc = it - 3
                # late chunks run with no proj/attention stages left, so
                # their oproj can also cycle through the idle psA (and,
                # for the final chunk, psC) banks — deeper PSUM pipeline
                # keeps the PE from stalling on PSUM->SBUF drains.
                if sc == NCH - 1:
                    opsum = [(psB, "omm"), (psA, "mm"), (psC, "score")]
                elif sc == NCH - 2:
                    opsum = [(psB, "omm"), (psA, "mm")]
                else:
                    opsum = [(psB, "omm")]
                pi = 0
                for st in range(4 * sc, 4 * sc + 4):
                    # whole output row block [128, HID] is staged in one
                    # SBUF tile and stored with a single large DMA
                    osb = opool.tile([128, HID], BF16, tag="osb")
                    for jp in range(HID // SC_ATT // 2):  # jc pairs
                        pool_, tag_ = opsum[pi % len(opsum)]
                        pi += 1
                        opsa = pool_.tile([128, SC_ATT], FP32, tag=tag_)
                        opsb = pool_.tile([128, SC_ATT], FP32, tag=tag_)
                        # jc pair shares each head's lhsT (one LDWEIGHTS
                        # per head instead of per matmul)
                        for h in range(2):
                            for ops, jc in ((opsa, 2 * jp),
                                            (opsb, 2 * jp + 1)):
                                nc.tensor.matmul(
                                    ops,
                                    lhsT=attnT[h][:, bass.ts(st, 128)],
                                    rhs=woTs[:, h, bass.ts(jc, SC_ATT)],
                                    start=(h == 0), stop=(h == 1))
                        nc.vector.tensor_copy(
                            osb[:, bass.ts(2 * jp, SC_ATT)], opsa)
                        nc.scalar.copy(
                            osb[:, bass.ts(2 * jp + 1, SC_ATT)], opsb)
                    if sc == NCH - 1 and st == 4 * sc + 3:
                        # very last row block: halves on both rings so the
                        # final store drain is not serialized on one ring
                        nc.sync.dma_start(
                            out=outp[bass.ts(st, 128), 0:HID // 2],
                            in_=osb[:, 0:HID // 2])
                        nc.scalar.dma_start(
                            out=outp[bass.ts(st, 128), HID // 2:HID],
                            in_=osb[:, HID // 2:HID])
                    else:
                        eng = nc.sync if st % 2 == 0 else nc.scalar
                        eng.dma_start(out=outp[bass.ts(st, 128), :],
                                      in_=osb)

    nc.compile()
    return nc


def _prep_inputs(hidden_states, cos, sin, wqkv, bqkv, wo, q_norm_w, k_norm_w):
    """Host-side layout prep + per-core sharding. All device tensors are
    pre-swizzled so every DMA has long contiguous per-partition runs."""
    import ml_dtypes
    bf16 = ml_dtypes.bfloat16
    f32 = np.float32
    hTn = np.ascontiguousarray(hidden_states.reshape(S, HID).T).astype(bf16)
    hTh = np.ascontiguousarray(
        hTn.reshape(16, 128, S // SC_ATT, SC_ATT).transpose(1, 2, 0, 3)
    )  # [p, sc, kt, s] — chunk-major so chunk loads are contiguous
    cosT = cos.T.astype(f32)  # [64, S]
    sinT = sin.T.astype(f32)
    cs2 = np.concatenate([cosT, cosT], axis=0)  # [128, S]
    ss2 = np.concatenate([sinT, sinT], axis=0)
    qwv = q_norm_w.astype(f32).reshape(128, 1)
    kwv = k_norm_w.astype(f32).reshape(128, 1)
    qwsv = np.concatenate([q_norm_w[64:], q_norm_w[:64]]).astype(
        f32).reshape(128, 1)
    kwsv = np.concatenate([k_norm_w[64:], k_norm_w[:64]]).astype(
        f32).reshape(128, 1)
    qc2 = np.ascontiguousarray(qwv * cs2).astype(bf16)
    qs2 = np.ascontiguousarray(qwsv * ss2).astype(bf16)
    kc2 = np.ascontiguousarray(kwv * cs2).astype(bf16)
    ks2 = np.ascontiguousarray(kwsv * ss2).astype(bf16)
    ones_np = np.ones((128, 128), dtype=f32)
    onesb_np = np.ones((128, 128), dtype=bf16)
    rt = np.zeros((128, 128), dtype=f32)
    rt[np.arange(64) + 64, np.arange(64)] = -1.0   # R^T[d+64, d] = -1
    rt[np.arange(64), np.arange(64) + 64] = 1.0    # R^T[d-64, d] = +1
    rt = rt.astype(bf16)
    # causal band masks: mk[p, j, f] = 1 iff query offset f >= key p + 128j
    pp = np.arange(128)[:, None]
    ff = np.arange(SC_ATT)[None, :]
    mk = np.stack([(ff >= pp + 128 * j) for j in range(4)],
                  axis=1).astype(bf16)  # [128, 4, 512]

    in_maps = []
    for c in range(N_CORES):
        kvh = c // 2
        rows = list(range(2 * c * HD, (2 * c + 2) * HD))          # q0, q1
        rows += list(range(NH * HD + kvh * HD, NH * HD + (kvh + 1) * HD))  # k
        rows += list(range((NH + NKV) * HD + kvh * HD,
                           (NH + NKV) * HD + (kvh + 1) * HD))      # v
        w_c = wqkv[rows]                       # [512, HID]
        wTc = np.ascontiguousarray(w_c.T).astype(f32)   # [HID, 512]
        wTk = wTc.reshape(16, 128, 512)
        b_c = bqkv[rows].astype(f32)           # [512]
        b4c = np.ascontiguousarray(b_c.reshape(4, 128).T)  # [128, 4]
        woc = wo[:, 2 * c * HD:(2 * c + 2) * HD]  # [HID, 256]
        woTc = np.ascontiguousarray(woc.T).astype(f32)  # [256, HID]
        woTh = np.ascontiguousarray(
            woTc.reshape(2, 128, HID).transpose(1, 0, 2)).astype(bf16)
        im = {
            "hT": hTh, "b4": b4c, "woT": woTh,
            "qc2": qc2, "qs2": qs2, "kc2": kc2, "ks2": ks2,
            "ones": ones_np, "onesb": onesb_np, "rswap": rt, "mk": mk,
        }
        for oc in range(4):
            im[f"wT{oc}"] = np.ascontiguousarray(
                wTk[:, :, oc * 128:(oc + 1) * 128].transpose(1, 0, 2)
            ).astype(bf16)
        in_maps.append(im)
    return in_maps


_NC_CACHE = {}


def kernel(hidden_states, cos, sin, k_cache, v_cache, mask,
           wqkv, bqkv, wo, bo, q_norm_w, k_norm_w, kv_write_indices,
           trace=False):
    hidden_states = np.asarray(hidden_states, dtype=np.float32)
    in_maps = _prep_inputs(
        np.asarray(hidden_states), np.asarray(cos), np.asarray(sin),
        np.asarray(wqkv), np.asarray(bqkv), np.asarray(wo),
        np.asarray(q_norm_w), np.asarray(k_norm_w))

    if "nc" not in _NC_CACHE:
        _NC_CACHE["nc"] = build_nc()
    nc = _NC_CACHE["nc"]

    res = run_bass_kernel_spmd(nc, in_maps, core_ids=list(range(N_CORES)),
                               trace=trace)
    out = np.zeros((S, HID), np.float32)
    for rmap in res.results:
        out += np.asarray(rmap["outp"], dtype=np.float32)
    out += np.asarray(bo, dtype=np.float32)[None, :]
    if trace:
        kernel.last_results = res
    return out.reshape(1, S, HID)


# revision 28
# speedup vs baseline: 1.0114x; 1.0114x over previous
"""Trainium2 Bass kernel for nn_Attention_3736621547687.

B=1, S=2048, HID=2048, NH=16, NKV=4, HD=128 attention block:
qkv proj -> per-head RMSNorm(q,k) -> RoPE -> causal GQA attention -> o proj.

Sharding: tensor-parallel over heads across 8 cores. Core c owns q heads
{2c, 2c+1} and kv head c//2 (replicated across the pair of cores sharing it).
Each core computes a partial o-projection output; the host sums the 8
partials (Megatron-style row-parallel reduce) and adds the output bias.

Device-side layout trick: everything is computed in "transposed" orientation
(feature dim on partitions, sequence on the free dim) so that no on-chip
transposes of activations are needed:
  - host supplies hidden^T, wqkv_c^T, wo_c^T, cos/sin tiled to [128, S]
  - qkv proj emits q^T/k^T/v^T directly
  - scores are computed as scores^T [keys, queries]; softmax denominators are
    partition-dim sums obtained with an all-ones [128,128] matmul that also
    replicates the result across partitions (giving the broadcast for free)
  - exp() is fused with the 1/sqrt(qpa) scale on the scalar engine; causal
    masking = multiplying exp values by a 0/1 band mask on the vector engine
    (identical to the reference's additive -1e9 mask in fp32)
  - softmax max-subtraction is skipped: scores are ~N(0,1) after RMSNorm so
    exp() cannot overflow; mathematically identical to the reference.
All activations/weights are bf16 (fast weight load, 2x DVE modes, half the
HBM traffic); accumulations stay in fp32 PSUM.
"""

import numpy as np
from contextlib import ExitStack

import concourse.bass as bass
import concourse.bacc as bacc
import concourse.mybir as mybir
import concourse.tile as tile
from concourse.masks import make_identity
from concourse.bass_utils import run_bass_kernel_spmd

S = 2048
HID = 2048
NH = 16
NKV = 4
HD = 128
G = NH // NKV
SCALE = float(128.0 ** -0.5)  # query_pre_attn_scalar = 128
EPS = 1e-6

FP32 = mybir.dt.float32
F32R = mybir.dt.float32r
BF16 = mybir.dt.bfloat16
MULT = mybir.AluOpType.mult
AF = mybir.ActivationFunctionType

N_CORES = 8
SC_ATT = 512    # attention/oproj moving-dim chunk


def _patch_act_tables():
    """Force Ln and Exp onto the single combined activation-table set so the
    scalar engine never reloads tables when rms-norm and softmax interleave.
    Set ids must keep their positions, so competing sets are emptied rather
    than removed."""
    import concourse.hw_specs as hw_specs
    import concourse.bacc as bacc_mod
    orig = hw_specs.get_activation_tables

    def patched(module_arch):
        t = orig(module_arch)
        for name in ("exp_and_others", "natural_log", "exp_and_friends"):
            if name in t and "natural_log_exp_and_others" in t:
                t[name] = set()
        return t

    bacc_mod.get_activation_tables = patched


def build_nc():
    _patch_act_tables()
    nc = bacc.Bacc()

    NCH = S // SC_ATT
    hT = nc.dram_tensor("hT", [128, NCH, 16, SC_ATT], BF16,
                        kind="ExternalInput")
    wts_d = [nc.dram_tensor(f"wT{i}", [128, 16, HD], BF16, kind="ExternalInput")
             for i in range(4)]
    b4 = nc.dram_tensor("b4", [128, 4], FP32, kind="ExternalInput")
    woT = nc.dram_tensor("woT", [128, 2, HID], BF16, kind="ExternalInput")
    # rope tables with the rms-norm weight folded in per partition:
    # qc2 = qw*[cos;cos], qs2 = swap(qw)*[sin;sin], likewise for k
    qc2 = nc.dram_tensor("qc2", [128, S], BF16, kind="ExternalInput")
    qs2 = nc.dram_tensor("qs2", [128, S], BF16, kind="ExternalInput")
    kc2 = nc.dram_tensor("kc2", [128, S], BF16, kind="ExternalInput")
    ks2 = nc.dram_tensor("ks2", [128, S], BF16, kind="ExternalInput")
    onesd = nc.dram_tensor("ones", [128, 128], F32R, kind="ExternalInput")
    onesbd = nc.dram_tensor("onesb", [128, 128], BF16, kind="ExternalInput")
    rswapd = nc.dram_tensor("rswap", [128, 128], BF16, kind="ExternalInput")
    maskd = nc.dram_tensor("mk", [128, 4, SC_ATT], BF16, kind="ExternalInput")
    outp = nc.dram_tensor("outp", [S, HID], BF16, kind="ExternalOutput")

    with ExitStack() as ctx:
        tc = ctx.enter_context(tile.TileContext(nc))

        const = ctx.enter_context(tc.tile_pool(name="const", bufs=1))
        hpool = ctx.enter_context(tc.tile_pool(name="hpool", bufs=2))
        rawp = ctx.enter_context(tc.tile_pool(name="rawp", bufs=1))
        atp = ctx.enter_context(tc.tile_pool(name="atp", bufs=1))
        wpool = ctx.enter_context(tc.tile_pool(name="wpool", bufs=3))
        vpool = ctx.enter_context(tc.tile_pool(name="vpool", bufs=1))
        ppool = ctx.enter_context(tc.tile_pool(name="ppool", bufs=2))
        dpool = ctx.enter_context(tc.tile_pool(name="dpool", bufs=2))
        opool = ctx.enter_context(tc.tile_pool(name="opool", bufs=2))

        psA = ctx.enter_context(tc.tile_pool(name="psA", bufs=2, space="PSUM"))
        psB = ctx.enter_context(tc.tile_pool(name="psB", bufs=2, space="PSUM"))
        psC = ctx.enter_context(tc.tile_pool(name="psC", bufs=2, space="PSUM"))
        psN = ctx.enter_context(tc.tile_pool(name="psN", bufs=1, space="PSUM"))
        psO = ctx.enter_context(tc.tile_pool(name="psO", bufs=1, space="PSUM"))

        # ---- weight loads ride the scalar HWDGE ring so they overlap the
        # chunk-0 hidden-state load on the sync ring (k-head slice first)
        wts = [None] * 4
        wts[2] = const.tile([128, 16, HD], BF16, name="wts2", tag="wts2")
        nc.scalar.dma_start(out=wts[2], in_=wts_d[2][:, :, :])

        # ---- small constants on the SWDGE queue ------------------------
        ident = const.tile([128, 128], BF16)
        make_identity(nc, ident)
        ones128 = const.tile([128, 128], F32R)
        nc.gpsimd.dma_start(out=ones128, in_=onesd[:, :])
        onesb = const.tile([128, 128], BF16)
        nc.gpsimd.dma_start(out=onesb, in_=onesbd[:, :])
        Rm = const.tile([128, 128], BF16)
        nc.gpsimd.dma_start(out=Rm, in_=rswapd[:, :])
        b4s = const.tile([128, 4], FP32)
        nc.gpsimd.dma_start(out=b4s, in_=b4[:, :])
        kc2s = const.tile([128, S], BF16)
        nc.gpsimd.dma_start(out=kc2s, in_=kc2[:, :])
        ks2s = const.tile([128, S], BF16)
        nc.gpsimd.dma_start(out=ks2s, in_=ks2[:, :])
        qc2s = const.tile([128, S], BF16)
        nc.gpsimd.dma_start(out=qc2s, in_=qc2[:, :])
        qs2s = const.tile([128, S], BF16)
        nc.gpsimd.dma_start(out=qs2s, in_=qs2[:, :])
        epsc = const.tile([128, 1], FP32)
        nc.vector.memset(epsc, EPS)
        mks = const.tile([128, 4, SC_ATT], BF16)
        nc.gpsimd.dma_start(out=mks, in_=maskd[:, :, :])

        rawq = [rawp.tile([128, S], BF16, tag=f"raw{i}", name=f"raw{i}")
                for i in range(3)]
        qhat = [atp.tile([128, S], BF16, tag=f"qh{i}", name=f"qh{i}")
                for i in range(2)]
        khat = atp.tile([128, S], BF16, tag="kh")
        attnT = [atp.tile([128, S], BF16, tag=f"attnT{h}", name=f"attnT{h}")
                 for h in range(2)]
        vsb = vpool.tile([128, 16, HD], BF16, tag="vsb")
        heads = [
            (rawq[2], khat, kc2s, ks2s),
            (rawq[0], qhat[0], qc2s, qs2s),
            (rawq[1], qhat[1], qc2s, qs2s),
        ]
        raw3s = {}

        # ================================================================
        # Software-pipelined emission: stage lag guarantees every
        # instruction's inputs are a full pipeline iteration old, so no
        # engine stream ever blocks at a phase boundary.
        #   iter sc: proj(sc) | rope+V(sc-1) | attention(sc-2) | oproj(sc-3)
        # ================================================================
        for it in range(NCH + 3):
            # ---- stage 1: qkv projection ------------------------------
            if it < NCH:
                sc = it
                sl = bass.ts(sc, SC_ATT)
                htsA = hpool.tile([128, 8, SC_ATT], BF16, tag="htsA")
                nc.sync.dma_start(out=htsA, in_=hT[:, sc, 0:8, :])
                htsB = hpool.tile([128, 8, SC_ATT], BF16, tag="htsB")
                nc.scalar.dma_start(out=htsB, in_=hT[:, sc, 8:16, :])
                if sc == 0:
                    # remaining weights follow htsB on the scalar ring;
                    # woT rides the SWDGE queue (needed latest)
                    for oc in (0, 1, 3):
                        wt = const.tile([128, 16, HD], BF16,
                                        name=f"wts{oc}", tag=f"wts{oc}")
                        nc.scalar.dma_start(out=wt, in_=wts_d[oc][:, :, :])
                        wts[oc] = wt
                    woTs = const.tile([128, 2, HID], BF16)
                    nc.gpsimd.dma_start(out=woTs, in_=woT[:, :, :])
                for oc in (2, 0, 1, 3):
                    ps = psA.tile([128, SC_ATT], FP32, tag="mm")
                    for kt in range(16):
                        src_h = htsA if kt < 8 else htsB
                        nc.tensor.matmul(
                            ps, lhsT=wts[oc][:, kt, :],
                            rhs=src_h[:, kt % 8, :],
                            start=(kt == 0), stop=(kt == 15))
                    if oc == 3:
                        raw3 = wpool.tile([128, SC_ATT], BF16, tag="raw3",
                                          bufs=2)
                        nc.vector.tensor_scalar_add(raw3, ps,
                                                    b4s[:, oc:oc + 1])
                        raw3s[sc] = raw3
                    else:
                        nc.vector.tensor_scalar_add(
                            rawq[oc][:, sl], ps, b4s[:, oc:oc + 1])

            # ---- stage 2: rmsnorm + rope + V tiles --------------------
            if 1 <= it <= NCH:
                sc = it - 1
                sl = bass.ts(sc, SC_ATT)
                for raw, dst, wc2_, ws2_ in heads:
                    sq = wpool.tile([128, SC_ATT], BF16, tag="sq", bufs=2)
                    nc.vector.tensor_mul(sq, raw[:, sl], raw[:, sl])
                    ssum = psN.tile([128, SC_ATT], FP32, tag="norm")
                    nc.tensor.matmul(ssum, lhsT=onesb, rhs=sq,
                                     start=True, stop=True)
                    lnb = wpool.tile([128, SC_ATT], BF16, tag="lnb", bufs=2)
                    nc.scalar.activation(lnb, ssum, AF.Ln,
                                         scale=1.0 / HD, bias=epsc)
                    nc.scalar.activation(lnb, lnb, AF.Exp, scale=-0.5)
                    rtp = psN.tile([128, SC_ATT], FP32, tag="norm")
                    nc.tensor.matmul(rtp, lhsT=Rm, rhs=raw[:, sl],
                                     start=True, stop=True)
                    t1 = wpool.tile([128, SC_ATT], BF16, tag="tt", bufs=3)
                    nc.vector.tensor_mul(t1, raw[:, sl], wc2_[:, sl])
                    t2 = wpool.tile([128, SC_ATT], BF16, tag="tt", bufs=3)
                    nc.vector.tensor_mul(t2, rtp, ws2_[:, sl])
                    t3 = wpool.tile([128, SC_ATT], BF16, tag="tt", bufs=3)
                    nc.vector.tensor_add(t3, t1, t2)
                    nc.vector.tensor_mul(dst[:, sl], t3, lnb)
                raw3 = raw3s.pop(sc)
                for j in range(4):
                    tt = 4 * sc + j
                    vps = psN.tile([128, 128], BF16, tag="norm")
                    nc.tensor.transpose(vps,
                                        raw3[:, bass.ts(j, 128)], ident)
                    nc.vector.tensor_copy(vsb[:, tt, :], vps)

            # ---- stage 3: attention, both heads -----------------------
            # The last query chunk is processed in two 256-wide halves:
            # the first half skips the top key tiles entirely (less exp
            # work) and its o-projection overlaps the second half's
            # attention, shortening the serial tail of the kernel.
            if 2 <= it <= NCH + 1:
                sc = it - 2
                if sc < NCH - 1:
                    subs = [(sc * SC_ATT, SC_ATT)]
                else:
                    subs = [(sc * SC_ATT, SC_ATT // 2),
                            (sc * SC_ATT + SC_ATT // 2, SC_ATT // 2)]
                for q0, qn in subs:
                    for h in range(2):
                        ntt = (q0 + qn) // 128
                        outps = psO.tile([128, qn], FP32, tag="attnout")
                        dacc = dpool.tile([128, qn], F32R, tag="dacc")
                        t0 = 0
                        paired = qn * 2 * 4 <= 2048  # 2 tiles fit one bank
                        while t0 < ntt:
                            gn = min(4, ntt - t0)
                            pg = ppool.tile([128, 4, qn], BF16, tag="pt")
                            if paired:
                                # two key tiles share one PSUM bank and a
                                # single exp ACTIVATE (halves ACT op count)
                                for jp in range(0, gn, 2):
                                    scp2 = psC.tile([128, 2, qn], FP32,
                                                    tag="score")
                                    for u in range(2):
                                        tt = t0 + jp + u
                                        nc.tensor.matmul(
                                            scp2[:, u, :],
                                            lhsT=khat[:, bass.ts(tt, 128)],
                                            rhs=qhat[h][:, bass.ds(q0, qn)],
                                            start=True, stop=True)
                                    nc.scalar.activation(
                                        pg[:, jp:jp + 2, :], scp2,
                                        AF.Exp, scale=SCALE)
                                    for u in range(2):
                                        tt = t0 + jp + u
                                        j = jp + u
                                        jd = tt - q0 // 128
                                        if jd >= 0:
                                            nc.gpsimd.affine_select(
                                                out=pg[:, j, :],
                                                in_=pg[:, j, :],
                                                compare_op=(
                                                    mybir.AluOpType.is_ge),
                                                fill=0.0,
                                                base=q0 - tt * 128,
                                                channel_multiplier=-1,
                                                pattern=[[1, qn]])
                                        nc.tensor.matmul(
                                            outps, lhsT=vsb[:, tt, :],
                                            rhs=pg[:, j, :],
                                            start=(tt == 0),
                                            stop=(tt == ntt - 1))
                            else:
                                for j in range(gn):
                                    tt = t0 + j
                                    scp = psC.tile([128, qn], FP32,
                                                   tag="score")
                                    nc.tensor.matmul(
                                        scp, lhsT=khat[:, bass.ts(tt, 128)],
                                        rhs=qhat[h][:, bass.ds(q0, qn)],
                                        start=True, stop=True)
                                    nc.scalar.activation(pg[:, j, :], scp,
                                                         AF.Exp,
                                                         scale=SCALE)
                                    jd = tt - q0 // 128
                                    if jd >= 0:  # diagonal: zero t > s
                                        nc.gpsimd.affine_select(
                                            out=pg[:, j, :],
                                            in_=pg[:, j, :],
                                            compare_op=mybir.AluOpType.is_ge,
                                            fill=0.0,
                                            base=q0 - tt * 128,
                                            channel_multiplier=-1,
                                            pattern=[[1, qn]])
                                    nc.tensor.matmul(outps,
                                                     lhsT=vsb[:, tt, :],
                                                     rhs=pg[:, j, :],
                                                     start=(tt == 0),
                                                     stop=(tt == ntt - 1))
                            if gn == 4:
                                ga = ppool.tile([128, qn], BF16, tag="ga",
                                                bufs=2)
                                nc.vector.tensor_add(ga, pg[:, 0, :],
                                                     pg[:, 1, :])
                                gb = ppool.tile([128, qn], BF16, tag="gb",
                                                bufs=1)
                                nc.vector.tensor_add(gb, pg[:, 2, :],
                                                     pg[:, 3, :])
                                if t0 == 0:
                                    nc.vector.tensor_add(dacc, ga, gb)
                                else:
                                    gc = ppool.tile([128, qn], BF16,
                                                    tag="ga", bufs=2)
                                    nc.vector.tensor_add(gc, ga, gb)
                                    nc.vector.tensor_add(dacc, dacc, gc)
                            else:  # trailing pair (only in split chunks)
                                ga = ppool.tile([128, qn], BF16, tag="ga",
                                                bufs=2)
                                nc.vector.tensor_add(ga, pg[:, 0, :],
                                                     pg[:, 1, :])
                                nc.vector.tensor_add(dacc, dacc, ga)
                            t0 += gn
                        drep = psN.tile([128, qn], FP32, tag="norm")
                        nc.tensor.matmul(drep, lhsT=ones128, rhs=dacc,
                                         start=True, stop=True)
                        drec = wpool.tile([128, qn], FP32, tag="drec",
                                          bufs=2)
                        nc.vector.reciprocal_approx_fast(drec, drep)
                        nc.vector.tensor_mul(attnT[h][:, bass.ds(q0, qn)],
                                             outps, drec)

            # ---- stage 4: o projection --------------------------------
            if it >= 3:
                sc = it - 3
                # late chunks run with no proj/attention stages left, so
                # their oproj can also cycle through the idle psA (and,
                # for the final chunk, psC) banks — deeper PSUM pipeline
                # keeps the PE from stalling on PSUM->SBUF drains.
                if sc == NCH - 1:
                    opsum = [(psB, "omm"), (psA, "mm"), (psC, "score")]
                elif sc == NCH - 2:
                    opsum = [(psB, "omm"), (psA, "mm")]
                else:
                    opsum = [(psB, "omm")]
                pi = 0
                for st in range(4 * sc, 4 * sc + 4):
                    # whole output row block [128, HID] is staged in one
                    # SBUF tile and stored with a single large DMA
                    osb = opool.tile([128, HID], BF16, tag="osb")
                    for jp in range(HID // SC_ATT // 2):  # jc pairs
                        pool_, tag_ = opsum[pi % len(opsum)]
                        pi += 1
                        opsa = pool_.tile([128, SC_ATT], FP32, tag=tag_)
                        opsb = pool_.tile([128, SC_ATT], FP32, tag=tag_)
                        # jc pair shares each head's lhsT (one LDWEIGHTS
                        # per head instead of per matmul)
                        for h in range(2):
                            for ops, jc in ((opsa, 2 * jp),
                                            (opsb, 2 * jp + 1)):
                                nc.tensor.matmul(
                                    ops,
                                    lhsT=attnT[h][:, bass.ts(st, 128)],
                                    rhs=woTs[:, h, bass.ts(jc, SC_ATT)],
                                    start=(h == 0), stop=(h == 1))
                        nc.vector.tensor_copy(
                            osb[:, bass.ts(2 * jp, SC_ATT)], opsa)
                        nc.scalar.copy(
                            osb[:, bass.ts(2 * jp + 1, SC_ATT)], opsb)
                    if sc == NCH - 1 and st == 4 * sc + 3:
                        # very last row block: halves on both rings so the
                        # final store drain is not serialized on one ring
                        nc.sync.dma_start(
                            out=outp[bass.ts(st, 128), 0:HID // 2],
                            in_=osb[:, 0:HID // 2])
                        nc.scalar.dma_start(
                            out=outp[bass.ts(st, 128), HID // 2:HID],
                            in_=osb[:, HID // 2:HID])
                    else:
                        eng = nc.sync if st % 2 == 0 else nc.scalar
                        eng.dma_start(out=outp[bass.ts(st, 128), :],
                                      in_=osb)

    nc.compile()
    return nc


def _prep_inputs(hidden_states, cos, sin, wqkv, bqkv, wo, q_norm_w, k_norm_w):
    """Host-side layout prep + per-core sharding. All device tensors are
    pre-swizzled so every DMA has long contiguous per-partition runs."""
    import ml_dtypes
    bf16 = ml_dtypes.bfloat16
    f32 = np.float32
    hTn = np.ascontiguousarray(hidden_states.reshape(S, HID).T).astype(bf16)
    hTh = np.ascontiguousarray(
        hTn.reshape(16, 128, S // SC_ATT, SC_ATT).transpose(1, 2, 0, 3)
    )  # [p, sc, kt, s] — chunk-major so chunk loads are contiguous
    cosT = cos.T.astype(f32)  # [64, S]
    sinT = sin.T.astype(f32)
    cs2 = np.concatenate([cosT, cosT], axis=0)  # [128, S]
    ss2 = np.concatenate([sinT, sinT], axis=0)
    qwv = q_norm_w.astype(f32).reshape(128, 1)
    kwv = k_norm_w.astype(f32).reshape(128, 1)
    qwsv = np.concatenate([q_norm_w[64:], q_norm_w[:64]]).astype(
        f32).reshape(128, 1)
    kwsv = np.concatenate([k_norm_w[64:], k_norm_w[:64]]).astype(
        f32).reshape(128, 1)
    qc2 = np.ascontiguousarray(qwv * cs2).astype(bf16)
    qs2 = np.ascontiguousarray(qwsv * ss2).astype(bf16)
    kc2 = np.ascontiguousarray(kwv * cs2).astype(bf16)
    ks2 = np.ascontiguousarray(kwsv * ss2).astype(bf16)
    ones_np = np.ones((128, 128), dtype=f32)
    onesb_np = np.ones((128, 128), dtype=bf16)
    rt = np.zeros((128, 128), dtype=f32)
    rt[np.arange(64) + 64, np.arange(64)] = -1.0   # R^T[d+64, d] = -1
    rt[np.arange(64), np.arange(64) + 64] = 1.0    # R^T[d-64, d] = +1
    rt = rt.astype(bf16)
    # causal band masks: mk[p, j, f] = 1 iff query offset f >= key p + 128j
    pp = np.arange(128)[:, None]
    ff = np.arange(SC_ATT)[None, :]
    mk = np.stack([(ff >= pp + 128 * j) for j in range(4)],
                  axis=1).astype(bf16)  # [128, 4, 512]

    in_maps = []
    for c in range(N_CORES):
        kvh = c // 2
        rows = list(range(2 * c * HD, (2 * c + 2) * HD))          # q0, q1
        rows += list(range(NH * HD + kvh * HD, NH * HD + (kvh + 1) * HD))  # k
        rows += list(range((NH + NKV) * HD + kvh * HD,
                           (NH + NKV) * HD + (kvh + 1) * HD))      # v
        w_c = wqkv[rows]                       # [512, HID]
        wTc = np.ascontiguousarray(w_c.T).astype(f32)   # [HID, 512]
        wTk = wTc.reshape(16, 128, 512)
        b_c = bqkv[rows].astype(f32)           # [512]
        b4c = np.ascontiguousarray(b_c.reshape(4, 128).T)  # [128, 4]
        woc = wo[:, 2 * c * HD:(2 * c + 2) * HD]  # [HID, 256]
        woTc = np.ascontiguousarray(woc.T).astype(f32)  # [256, HID]
        woTh = np.ascontiguousarray(
            woTc.reshape(2, 128, HID).transpose(1, 0, 2)).astype(bf16)
        im = {
            "hT": hTh, "b4": b4c, "woT": woTh,
            "qc2": qc2, "qs2": qs2, "kc2": kc2, "ks2": ks2,
            "ones": ones_np, "onesb": onesb_np, "rswap": rt, "mk": mk,
        }
        for oc in range(4):
            im[f"wT{oc}"] = np.ascontiguousarray(
                wTk[:, :, oc * 128:(oc + 1) * 128].transpose(1, 0, 2)
            ).astype(bf16)
        in_maps.append(im)
    return in_maps


_NC_CACHE = {}


def kernel(hidden_states, cos, sin, k_cache, v_cache, mask,
           wqkv, bqkv, wo, bo, q_norm_w, k_norm_w, kv_write_indices,
           trace=False):
    hidden_states = np.asarray(hidden_states, dtype=np.float32)
    in_maps = _prep_inputs(
        np.asarray(hidden_states), np.asarray(cos), np.asarray(sin),
        np.asarray(wqkv), np.asarray(bqkv), np.asarray(wo),
        np.asarray(q_norm_w), np.asarray(k_norm_w))

    if "nc" not in _NC_CACHE:
        _NC_CACHE["nc"] = build_nc()
    nc = _NC_CACHE["nc"]

    res = run_bass_kernel_spmd(nc, in_maps, core_ids=list(range(N_CORES)),
                               trace=trace)
    out = np.zeros((S, HID), np.float32)
    for rmap in res.results:
        out += np.asarray(rmap["outp"], dtype=np.float32)
    out += np.asarray(bo, dtype=np.float32)[None, :]
    if trace:
        kernel.last_results = res
    return out.reshape(1, S, HID)


# revision 30
# speedup vs baseline: 1.0282x; 1.0166x over previous
"""Trainium2 Bass kernel for nn_Attention_3736621547687.

B=1, S=2048, HID=2048, NH=16, NKV=4, HD=128 attention block:
qkv proj -> per-head RMSNorm(q,k) -> RoPE -> causal GQA attention -> o proj.

Sharding: tensor-parallel over heads across 8 cores. Core c owns q heads
{2c, 2c+1} and kv head c//2 (replicated across the pair of cores sharing it).
Each core computes a partial o-projection output; the host sums the 8
partials (Megatron-style row-parallel reduce) and adds the output bias.

Device-side layout trick: everything is computed in "transposed" orientation
(feature dim on partitions, sequence on the free dim) so that no on-chip
transposes of activations are needed:
  - host supplies hidden^T, wqkv_c^T, wo_c^T, cos/sin tiled to [128, S]
  - qkv proj emits q^T/k^T/v^T directly
  - scores are computed as scores^T [keys, queries]; softmax denominators are
    partition-dim sums obtained with an all-ones [128,128] matmul that also
    replicates the result across partitions (giving the broadcast for free)
  - exp() is fused with the 1/sqrt(qpa) scale on the scalar engine; causal
    masking = multiplying exp values by a 0/1 band mask on the vector engine
    (identical to the reference's additive -1e9 mask in fp32)
  - softmax max-subtraction is skipped: scores are ~N(0,1) after RMSNorm so
    exp() cannot overflow; mathematically identical to the reference.
All activations/weights are bf16 (fast weight load, 2x DVE modes, half the
HBM traffic); accumulations stay in fp32 PSUM.
"""

import numpy as np
from contextlib import ExitStack

import concourse.bass as bass
import concourse.bacc as bacc
import concourse.mybir as mybir
import concourse.tile as tile
from concourse.masks import make_identity
from concourse.bass_utils import run_bass_kernel_spmd

S = 2048
HID = 2048
NH = 16
NKV = 4
HD = 128
G = NH // NKV
SCALE = float(128.0 ** -0.5)  # query_pre_attn_scalar = 128
EPS = 1e-6

FP32 = mybir.dt.float32
F32R = mybir.dt.float32r
BF16 = mybir.dt.bfloat16
MULT = mybir.AluOpType.mult
AF = mybir.ActivationFunctionType

N_CORES = 8
SC_ATT = 512    # attention/oproj moving-dim chunk


def _patch_act_tables():
    """Force Ln and Exp onto the single combined activation-table set so the
    scalar engine never reloads tables when rms-norm and softmax interleave.
    Set ids must keep their positions, so competing sets are emptied rather
    than removed."""
    import concourse.hw_specs as hw_specs
    import concourse.bacc as bacc_mod
    orig = hw_specs.get_activation_tables

    def patched(module_arch):
        t = orig(module_arch)
        for name in ("exp_and_others", "natural_log", "exp_and_friends"):
            if name in t and "natural_log_exp_and_others" in t:
                t[name] = set()
        return t

    bacc_mod.get_activation_tables = patched


def build_nc():
    _patch_act_tables()
    nc = bacc.Bacc()

    NCH = S // SC_ATT
    hT = nc.dram_tensor("hT", [128, NCH, 16, SC_ATT], BF16,
                        kind="ExternalInput")
    wts_d = [nc.dram_tensor(f"wT{i}", [128, 16, HD], BF16, kind="ExternalInput")
             for i in range(4)]
    b4 = nc.dram_tensor("b4", [128, 4], FP32, kind="ExternalInput")
    woT = nc.dram_tensor("woT", [128, 2, HID], BF16, kind="ExternalInput")
    # rope tables with the rms-norm weight folded in per partition:
    # qc2 = qw*[cos;cos], qs2 = swap(qw)*[sin;sin], likewise for k
    qc2 = nc.dram_tensor("qc2", [128, S], BF16, kind="ExternalInput")
    qs2 = nc.dram_tensor("qs2", [128, S], BF16, kind="ExternalInput")
    kc2 = nc.dram_tensor("kc2", [128, S], BF16, kind="ExternalInput")
    ks2 = nc.dram_tensor("ks2", [128, S], BF16, kind="ExternalInput")
    onesd = nc.dram_tensor("ones", [128, 128], F32R, kind="ExternalInput")
    onesbd = nc.dram_tensor("onesb", [128, 128], BF16, kind="ExternalInput")
    rswapd = nc.dram_tensor("rswap", [128, 128], BF16, kind="ExternalInput")
    maskd = nc.dram_tensor("mk", [128, 4, SC_ATT], BF16, kind="ExternalInput")
    outp = nc.dram_tensor("outp", [S, HID], BF16, kind="ExternalOutput")

    with ExitStack() as ctx:
        tc = ctx.enter_context(tile.TileContext(nc))

        const = ctx.enter_context(tc.tile_pool(name="const", bufs=1))
        hpool = ctx.enter_context(tc.tile_pool(name="hpool", bufs=2))
        rawp = ctx.enter_context(tc.tile_pool(name="rawp", bufs=1))
        atp = ctx.enter_context(tc.tile_pool(name="atp", bufs=1))
        wpool = ctx.enter_context(tc.tile_pool(name="wpool", bufs=3))
        vpool = ctx.enter_context(tc.tile_pool(name="vpool", bufs=1))
        ppool = ctx.enter_context(tc.tile_pool(name="ppool", bufs=2))
        dpool = ctx.enter_context(tc.tile_pool(name="dpool", bufs=2))
        opool = ctx.enter_context(tc.tile_pool(name="opool", bufs=2))

        psA = ctx.enter_context(tc.tile_pool(name="psA", bufs=2, space="PSUM"))
        psB = ctx.enter_context(tc.tile_pool(name="psB", bufs=2, space="PSUM"))
        psC = ctx.enter_context(tc.tile_pool(name="psC", bufs=2, space="PSUM"))
        psN = ctx.enter_context(tc.tile_pool(name="psN", bufs=1, space="PSUM"))
        psO = ctx.enter_context(tc.tile_pool(name="psO", bufs=1, space="PSUM"))

        # ---- weight loads ride the scalar HWDGE ring so they overlap the
        # chunk-0 hidden-state load on the sync ring (k-head slice first)
        wts = [None] * 4
        wts[2] = const.tile([128, 16, HD], BF16, name="wts2", tag="wts2")
        nc.scalar.dma_start(out=wts[2], in_=wts_d[2][:, :, :])

        # ---- small constants on the SWDGE queue ------------------------
        ident = const.tile([128, 128], BF16)
        make_identity(nc, ident)
        ones128 = const.tile([128, 128], F32R)
        nc.gpsimd.dma_start(out=ones128, in_=onesd[:, :])
        onesb = const.tile([128, 128], BF16)
        nc.gpsimd.dma_start(out=onesb, in_=onesbd[:, :])
        Rm = const.tile([128, 128], BF16)
        nc.gpsimd.dma_start(out=Rm, in_=rswapd[:, :])
        b4s = const.tile([128, 4], FP32)
        nc.gpsimd.dma_start(out=b4s, in_=b4[:, :])
        kc2s = const.tile([128, S], BF16)
        nc.gpsimd.dma_start(out=kc2s, in_=kc2[:, :])
        ks2s = const.tile([128, S], BF16)
        nc.gpsimd.dma_start(out=ks2s, in_=ks2[:, :])
        qc2s = const.tile([128, S], BF16)
        nc.gpsimd.dma_start(out=qc2s, in_=qc2[:, :])
        qs2s = const.tile([128, S], BF16)
        nc.gpsimd.dma_start(out=qs2s, in_=qs2[:, :])
        epsc = const.tile([128, 1], FP32)
        nc.vector.memset(epsc, EPS)
        mks = const.tile([128, 4, SC_ATT], BF16)
        nc.gpsimd.dma_start(out=mks, in_=maskd[:, :, :])

        rawq = [rawp.tile([128, S], BF16, tag=f"raw{i}", name=f"raw{i}")
                for i in range(3)]
        qhat = [atp.tile([128, S], BF16, tag=f"qh{i}", name=f"qh{i}")
                for i in range(2)]
        khat = atp.tile([128, S], BF16, tag="kh")
        attnT = [atp.tile([128, S], BF16, tag=f"attnT{h}", name=f"attnT{h}")
                 for h in range(2)]
        vsb = vpool.tile([128, 16, HD], BF16, tag="vsb")
        heads = [
            (rawq[2], khat, kc2s, ks2s),
            (rawq[0], qhat[0], qc2s, qs2s),
            (rawq[1], qhat[1], qc2s, qs2s),
        ]
        raw3s = {}

        # ================================================================
        # Software-pipelined emission: stage lag guarantees every
        # instruction's inputs are a full pipeline iteration old, so no
        # engine stream ever blocks at a phase boundary.
        #   iter sc: proj(sc) | rope+V(sc-1) | attention(sc-2) | oproj(sc-3)
        # ================================================================
        for it in range(NCH + 3):
            # ---- stage 1: qkv projection ------------------------------
            if it < NCH:
                sc = it
                sl = bass.ts(sc, SC_ATT)
                htsA = hpool.tile([128, 8, SC_ATT], BF16, tag="htsA")
                nc.sync.dma_start(out=htsA, in_=hT[:, sc, 0:8, :])
                htsB = hpool.tile([128, 8, SC_ATT], BF16, tag="htsB")
                nc.scalar.dma_start(out=htsB, in_=hT[:, sc, 8:16, :])
                if sc == 0:
                    # remaining weights follow htsB on the scalar ring;
                    # woT rides the SWDGE queue (needed latest)
                    for oc in (0, 1, 3):
                        wt = const.tile([128, 16, HD], BF16,
                                        name=f"wts{oc}", tag=f"wts{oc}")
                        nc.scalar.dma_start(out=wt, in_=wts_d[oc][:, :, :])
                        wts[oc] = wt
                    woTs = const.tile([128, 2, HID], BF16)
                    nc.gpsimd.dma_start(out=woTs, in_=woT[:, :, :])
                for oc in (2, 0, 1, 3):
                    ps = psA.tile([128, SC_ATT], FP32, tag="mm")
                    for kt in range(16):
                        src_h = htsA if kt < 8 else htsB
                        nc.tensor.matmul(
                            ps, lhsT=wts[oc][:, kt, :],
                            rhs=src_h[:, kt % 8, :],
                            start=(kt == 0), stop=(kt == 15))
                    if oc == 3:
                        raw3 = wpool.tile([128, SC_ATT], BF16, tag="raw3",
                                          bufs=2)
                        nc.vector.tensor_scalar_add(raw3, ps,
                                                    b4s[:, oc:oc + 1])
                        raw3s[sc] = raw3
                    else:
                        nc.vector.tensor_scalar_add(
                            rawq[oc][:, sl], ps, b4s[:, oc:oc + 1])

            # ---- stage 2: rmsnorm + rope + V tiles --------------------
            if 1 <= it <= NCH:
                sc = it - 1
                sl = bass.ts(sc, SC_ATT)
                for raw, dst, wc2_, ws2_ in heads:
                    sq = wpool.tile([128, SC_ATT], BF16, tag="sq", bufs=2)
                    nc.vector.tensor_mul(sq, raw[:, sl], raw[:, sl])
                    ssum = psN.tile([128, SC_ATT], FP32, tag="norm")
                    nc.tensor.matmul(ssum, lhsT=onesb, rhs=sq,
                                     start=True, stop=True)
                    lnb = wpool.tile([128, SC_ATT], BF16, tag="lnb", bufs=2)
                    nc.scalar.activation(lnb, ssum, AF.Ln,
                                         scale=1.0 / HD, bias=epsc)
                    nc.scalar.activation(lnb, lnb, AF.Exp, scale=-0.5)
                    rtp = psN.tile([128, SC_ATT], FP32, tag="norm")
                    nc.tensor.matmul(rtp, lhsT=Rm, rhs=raw[:, sl],
                                     start=True, stop=True)
                    t1 = wpool.tile([128, SC_ATT], BF16, tag="tt", bufs=3)
                    nc.vector.tensor_mul(t1, raw[:, sl], wc2_[:, sl])
                    t2 = wpool.tile([128, SC_ATT], BF16, tag="tt", bufs=3)
                    nc.vector.tensor_mul(t2, rtp, ws2_[:, sl])
                    t3 = wpool.tile([128, SC_ATT], BF16, tag="tt", bufs=3)
                    nc.vector.tensor_add(t3, t1, t2)
                    nc.vector.tensor_mul(dst[:, sl], t3, lnb)
                raw3 = raw3s.pop(sc)
                for j in range(4):
                    tt = 4 * sc + j
                    vps = psN.tile([128, 128], BF16, tag="norm")
                    nc.tensor.transpose(vps,
                                        raw3[:, bass.ts(j, 128)], ident)
                    nc.vector.tensor_copy(vsb[:, tt, :], vps)

            # ---- stage 3: attention, both heads -----------------------
            # The last query chunk is processed in two 256-wide halves:
            # the first half skips the top key tiles entirely (less exp
            # work) and its o-projection overlaps the second half's
            # attention, shortening the serial tail of the kernel.
            if 2 <= it <= NCH + 1:
                sc = it - 2
                if sc < NCH - 1:
                    subs = [(sc * SC_ATT, SC_ATT)]
                else:
                    subs = [(sc * SC_ATT, SC_ATT // 2),
                            (sc * SC_ATT + SC_ATT // 2, SC_ATT // 2)]
                for q0, qn in subs:
                    for h in range(2):
                        ntt = (q0 + qn) // 128
                        outps = psO.tile([128, qn], FP32, tag="attnout")
                        dacc = dpool.tile([128, qn], F32R, tag="dacc")
                        t0 = 0
                        paired = qn * 2 * 4 <= 2048  # 2 tiles fit one bank
                        while t0 < ntt:
                            gn = min(4, ntt - t0)
                            pg = ppool.tile([128, 4, qn], BF16, tag="pt")
                            if paired:
                                # two key tiles share one PSUM bank and a
                                # single exp ACTIVATE (halves ACT op count)
                                for jp in range(0, gn, 2):
                                    scp2 = psC.tile([128, 2, qn], FP32,
                                                    tag="score")
                                    for u in range(2):
                                        tt = t0 + jp + u
                                        nc.tensor.matmul(
                                            scp2[:, u, :],
                                            lhsT=khat[:, bass.ts(tt, 128)],
                                            rhs=qhat[h][:, bass.ds(q0, qn)],
                                            start=True, stop=True)
                                    nc.scalar.activation(
                                        pg[:, jp:jp + 2, :], scp2,
                                        AF.Exp, scale=SCALE)
                                    for u in range(2):
                                        tt = t0 + jp + u
                                        j = jp + u
                                        jd = tt - q0 // 128
                                        if jd >= 0:
                                            nc.vector.tensor_mul(
                                                pg[:, j, :], pg[:, j, :],
                                                mks[:, jd, 0:qn])
                                        nc.tensor.matmul(
                                            outps, lhsT=vsb[:, tt, :],
                                            rhs=pg[:, j, :],
                                            start=(tt == 0),
                                            stop=(tt == ntt - 1))
                            else:
                                for j in range(gn):
                                    tt = t0 + j
                                    scp = psC.tile([128, qn], FP32,
                                                   tag="score")
                                    nc.tensor.matmul(
                                        scp, lhsT=khat[:, bass.ts(tt, 128)],
                                        rhs=qhat[h][:, bass.ds(q0, qn)],
                                        start=True, stop=True)
                                    nc.scalar.activation(pg[:, j, :], scp,
                                                         AF.Exp,
                                                         scale=SCALE)
                                    jd = tt - q0 // 128
                                    if jd >= 0:  # diagonal: zero t > s
                                        nc.vector.tensor_mul(
                                            pg[:, j, :], pg[:, j, :],
                                            mks[:, jd, 0:qn])
                                    nc.tensor.matmul(outps,
                                                     lhsT=vsb[:, tt, :],
                                                     rhs=pg[:, j, :],
                                                     start=(tt == 0),
                                                     stop=(tt == ntt - 1))
                            if gn == 4:
                                ga = ppool.tile([128, qn], BF16, tag="ga",
                                                bufs=2)
                                nc.vector.tensor_add(ga, pg[:, 0, :],
                                                     pg[:, 1, :])
                                gb = ppool.tile([128, qn], BF16, tag="gb",
                                                bufs=1)
                                nc.vector.tensor_add(gb, pg[:, 2, :],
                                                     pg[:, 3, :])
                                if t0 == 0:
                                    nc.vector.tensor_add(dacc, ga, gb)
                                else:
                                    gc = ppool.tile([128, qn], BF16,
                                                    tag="ga", bufs=2)
                                    nc.vector.tensor_add(gc, ga, gb)
                                    nc.vector.tensor_add(dacc, dacc, gc)
                            else:  # trailing pair (only in split chunks)
                                ga = ppool.tile([128, qn], BF16, tag="ga",
                                                bufs=2)
                                nc.vector.tensor_add(ga, pg[:, 0, :],
                                                     pg[:, 1, :])
                                nc.vector.tensor_add(dacc, dacc, ga)
                            t0 += gn
                        drep = psN.tile([128, qn], FP32, tag="norm")
                        nc.tensor.matmul(drep, lhsT=ones128, rhs=dacc,
                                         start=True, stop=True)
                        drec = wpool.tile([128, qn], FP32, tag="drec",
                                          bufs=2)
                        nc.vector.reciprocal_approx_fast(drec, drep)
                        nc.vector.tensor_mul(attnT[h][:, bass.ds(q0, qn)],
                                             outps, drec)

            # ---- stage 4: o projection --------------------------------
            if it >= 3:
                sc = it - 3
                # late chunks run with no proj/attention stages left, so
                # their oproj can also cycle through the idle psA (and,
                # for the final chunk, psC) banks — deeper PSUM pipeline
                # keeps the PE from stalling on PSUM->SBUF drains.
                if sc == NCH - 1:
                    opsum = [(psB, "omm"), (psA, "mm"), (psC, "score")]
                elif sc == NCH - 2:
                    opsum = [(psB, "omm"), (psA, "mm")]
                else:
                    opsum = [(psB, "omm")]
                pi = 0
                for st in range(4 * sc, 4 * sc + 4):
                    # whole output row block [128, HID] is staged in one
                    # SBUF tile and stored with a single large DMA
                    osb = opool.tile([128, HID], BF16, tag="osb")
                    for jp in range(HID // SC_ATT // 2):  # jc pairs
                        pool_, tag_ = opsum[pi % len(opsum)]
                        pi += 1
                        opsa = pool_.tile([128, SC_ATT], FP32, tag=tag_)
                        opsb = pool_.tile([128, SC_ATT], FP32, tag=tag_)
                        # jc pair shares each head's lhsT (one LDWEIGHTS
                        # per head instead of per matmul)
                        for h in range(2):
                            for ops, jc in ((opsa, 2 * jp),
                                            (opsb, 2 * jp + 1)):
                                nc.tensor.matmul(
                                    ops,
                                    lhsT=attnT[h][:, bass.ts(st, 128)],
                                    rhs=woTs[:, h, bass.ts(jc, SC_ATT)],
                                    start=(h == 0), stop=(h == 1))
                        nc.vector.tensor_copy(
                            osb[:, bass.ts(2 * jp, SC_ATT)], opsa)
                        nc.scalar.copy(
                            osb[:, bass.ts(2 * jp + 1, SC_ATT)], opsb)
                    if sc == NCH - 1 and st == 4 * sc + 3:
                        # very last row block: halves on both rings so the
                        # final store drain is not serialized on one ring
                        nc.sync.dma_start(
                            out=outp[bass.ts(st, 128), 0:HID // 2],
                            in_=osb[:, 0:HID // 2])
                        nc.scalar.dma_start(
                            out=outp[bass.ts(st, 128), HID // 2:HID],
                            in_=osb[:, HID // 2:HID])
                    else:
                        eng = nc.sync if st % 2 == 0 else nc.scalar
                        eng.dma_start(out=outp[bass.ts(st, 128), :],
                                      in_=osb)

    nc.compile()
    return nc


def _prep_inputs(hidden_states, cos, sin, wqkv, bqkv, wo, q_norm_w, k_norm_w):
    """Host-side layout prep + per-core sharding. All device tensors are
    pre-swizzled so every DMA has long contiguous per-partition runs."""
    import ml_dtypes
    bf16 = ml_dtypes.bfloat16
    f32 = np.float32
    hTn = np.ascontiguousarray(hidden_states.reshape(S, HID).T).astype(bf16)
    hTh = np.ascontiguousarray(
        hTn.reshape(16, 128, S // SC_ATT, SC_ATT).transpose(1, 2, 0, 3)
    )  # [p, sc, kt, s] — chunk-major so chunk loads are contiguous
    cosT = cos.T.astype(f32)  # [64, S]
    sinT = sin.T.astype(f32)
    cs2 = np.concatenate([cosT, cosT], axis=0)  # [128, S]
    ss2 = np.concatenate([sinT, sinT], axis=0)
    qwv = q_norm_w.astype(f32).reshape(128, 1)
    kwv = k_norm_w.astype(f32).reshape(128, 1)
    qwsv = np.concatenate([q_norm_w[64:], q_norm_w[:64]]).astype(
        f32).reshape(128, 1)
    kwsv = np.concatenate([k_norm_w[64:], k_norm_w[:64]]).astype(
        f32).reshape(128, 1)
    qc2 = np.ascontiguousarray(qwv * cs2).astype(bf16)
    qs2 = np.ascontiguousarray(qwsv * ss2).astype(bf16)
    kc2 = np.ascontiguousarray(kwv * cs2).astype(bf16)
    ks2 = np.ascontiguousarray(kwsv * ss2).astype(bf16)
    ones_np = np.ones((128, 128), dtype=f32)
    onesb_np = np.ones((128, 128), dtype=bf16)
    rt = np.zeros((128, 128), dtype=f32)
    rt[np.arange(64) + 64, np.arange(64)] = -1.0   # R^T[d+64, d] = -1
    rt[np.arange(64), np.arange(64) + 64] = 1.0    # R^T[d-64, d] = +1
    rt = rt.astype(bf16)
    # causal band masks: mk[p, j, f] = 1 iff query offset f >= key p + 128j
    pp = np.arange(128)[:, None]
    ff = np.arange(SC_ATT)[None, :]
    mk = np.stack([(ff >= pp + 128 * j) for j in range(4)],
                  axis=1).astype(bf16)  # [128, 4, 512]

    in_maps = []
    for c in range(N_CORES):
        kvh = c // 2
        rows = list(range(2 * c * HD, (2 * c + 2) * HD))          # q0, q1
        rows += list(range(NH * HD + kvh * HD, NH * HD + (kvh + 1) * HD))  # k
        rows += list(range((NH + NKV) * HD + kvh * HD,
                           (NH + NKV) * HD + (kvh + 1) * HD))      # v
        w_c = wqkv[rows]                       # [512, HID]
        wTc = np.ascontiguousarray(w_c.T).astype(f32)   # [HID, 512]
        wTk = wTc.reshape(16, 128, 512)
        b_c = bqkv[rows].astype(f32)           # [512]
        b4c = np.ascontiguousarray(b_c.reshape(4, 128).T)  # [128, 4]
        woc = wo[:, 2 * c * HD:(2 * c + 2) * HD]  # [HID, 256]
        woTc = np.ascontiguousarray(woc.T).astype(f32)  # [256, HID]
        woTh = np.ascontiguousarray(
            woTc.reshape(2, 128, HID).transpose(1, 0, 2)).astype(bf16)
        im = {
            "hT": hTh, "b4": b4c, "woT": woTh,
            "qc2": qc2, "qs2": qs2, "kc2": kc2, "ks2": ks2,
            "ones": ones_np, "onesb": onesb_np, "rswap": rt, "mk": mk,
        }
        for oc in range(4):
            im[f"wT{oc}"] = np.ascontiguousarray(
                wTk[:, :, oc * 128:(oc + 1) * 128].transpose(1, 0, 2)
            ).astype(bf16)
        in_maps.append(im)
    return in_maps


_NC_CACHE = {}


def kernel(hidden_states, cos, sin, k_cache, v_cache, mask,
           wqkv, bqkv, wo, bo, q_norm_w, k_norm_w, kv_write_indices,
           trace=False):
    hidden_states = np.asarray(hidden_states, dtype=np.float32)
    in_maps = _prep_inputs(
        np.asarray(hidden_states), np.asarray(cos), np.asarray(sin),
        np.asarray(wqkv), np.asarray(bqkv), np.asarray(wo),
        np.asarray(q_norm_w), np.asarray(k_norm_w))

    if "nc" not in _NC_CACHE:
        _NC_CACHE["nc"] = build_nc()
    nc = _NC_CACHE["nc"]

    res = run_bass_kernel_spmd(nc, in_maps, core_ids=list(range(N_CORES)),
                               trace=trace)
    out = np.zeros((S, HID), np.float32)
    for rmap in res.results:
        out += np.asarray(rmap["outp"], dtype=np.float32)
    out += np.asarray(bo, dtype=np.float32)[None, :]
    if trace:
        kernel.last_results = res
    return out.reshape(1, S, HID)


# revision 34
# speedup vs baseline: 1.0873x; 1.0575x over previous
"""Trainium2 Bass kernel for nn_Attention_3736621547687.

B=1, S=2048, HID=2048, NH=16, NKV=4, HD=128 attention block:
qkv proj -> per-head RMSNorm(q,k) -> RoPE -> causal GQA attention -> o proj.

Sharding: tensor-parallel over heads across 8 cores. Core c owns q heads
{2c, 2c+1} and kv head c//2 (replicated across the pair of cores sharing it).
Each core computes a partial o-projection output; the host sums the 8
partials (Megatron-style row-parallel reduce) and adds the output bias.

Device-side layout trick: everything is computed in "transposed" orientation
(feature dim on partitions, sequence on the free dim) so that no on-chip
transposes of activations are needed:
  - host supplies hidden^T, wqkv_c^T, wo_c^T, cos/sin tiled to [128, S]
  - qkv proj emits q^T/k^T/v^T directly
  - scores are computed as scores^T [keys, queries]; softmax denominators are
    partition-dim sums obtained with an all-ones [128,128] matmul that also
    replicates the result across partitions (giving the broadcast for free)
  - exp() is fused with the 1/sqrt(qpa) scale on the scalar engine; causal
    masking = multiplying exp values by a 0/1 band mask on the vector engine
    (identical to the reference's additive -1e9 mask in fp32)
  - softmax max-subtraction is skipped: scores are ~N(0,1) after RMSNorm so
    exp() cannot overflow; mathematically identical to the reference.
All activations/weights are bf16 (fast weight load, 2x DVE modes, half the
HBM traffic); accumulations stay in fp32 PSUM.
"""

import numpy as np
from contextlib import ExitStack

import concourse.bass as bass
import concourse.bacc as bacc
import concourse.mybir as mybir
import concourse.tile as tile
from concourse.masks import make_identity
from concourse.bass_utils import run_bass_kernel_spmd

S = 2048
HID = 2048
NH = 16
NKV = 4
HD = 128
G = NH // NKV
SCALE = float(128.0 ** -0.5)  # query_pre_attn_scalar = 128
EPS = 1e-6

FP32 = mybir.dt.float32
F32R = mybir.dt.float32r
BF16 = mybir.dt.bfloat16
MULT = mybir.AluOpType.mult
AF = mybir.ActivationFunctionType

N_CORES = 8
SC_ATT = 512    # attention/oproj moving-dim chunk


def _patch_act_tables():
    """Force Ln and Exp onto the single combined activation-table set so the
    scalar engine never reloads tables when rms-norm and softmax interleave.
    Set ids must keep their positions, so competing sets are emptied rather
    than removed."""
    import concourse.hw_specs as hw_specs
    import concourse.bacc as bacc_mod
    orig = hw_specs.get_activation_tables

    def patched(module_arch):
        t = orig(module_arch)
        for name in ("exp_and_others", "natural_log", "exp_and_friends"):
            if name in t and "natural_log_exp_and_others" in t:
                t[name] = set()
        return t

    bacc_mod.get_activation_tables = patched


def build_nc():
    _patch_act_tables()
    nc = bacc.Bacc()

    NCH = S // SC_ATT
    hT = nc.dram_tensor("hT", [128, NCH, 16, SC_ATT], BF16,
                        kind="ExternalInput")
    wts_d = [nc.dram_tensor(f"wT{i}", [128, 16, HD], BF16, kind="ExternalInput")
             for i in range(4)]
    b4 = nc.dram_tensor("b4", [128, 4], FP32, kind="ExternalInput")
    woT = nc.dram_tensor("woT", [128, 2, HID], BF16, kind="ExternalInput")
    # rope tables with the rms-norm weight folded in per partition:
    # qc2 = qw*[cos;cos], qs2 = swap(qw)*[sin;sin], likewise for k
    qc2 = nc.dram_tensor("qc2", [128, S], BF16, kind="ExternalInput")
    qs2 = nc.dram_tensor("qs2", [128, S], BF16, kind="ExternalInput")
    kc2 = nc.dram_tensor("kc2", [128, S], BF16, kind="ExternalInput")
    ks2 = nc.dram_tensor("ks2", [128, S], BF16, kind="ExternalInput")
    onesd = nc.dram_tensor("ones", [128, 128], F32R, kind="ExternalInput")
    onesbd = nc.dram_tensor("onesb", [128, 128], BF16, kind="ExternalInput")
    rswapd = nc.dram_tensor("rswap", [128, 128], BF16, kind="ExternalInput")
    outp = nc.dram_tensor("outp", [S, HID], BF16, kind="ExternalOutput")

    with ExitStack() as ctx:
        tc = ctx.enter_context(tile.TileContext(nc))

        const = ctx.enter_context(tc.tile_pool(name="const", bufs=1))
        hpool = ctx.enter_context(tc.tile_pool(name="hpool", bufs=2))
        rawp = ctx.enter_context(tc.tile_pool(name="rawp", bufs=1))
        atp = ctx.enter_context(tc.tile_pool(name="atp", bufs=1))
        wpool = ctx.enter_context(tc.tile_pool(name="wpool", bufs=3))
        vpool = ctx.enter_context(tc.tile_pool(name="vpool", bufs=1))
        ppool = ctx.enter_context(tc.tile_pool(name="ppool", bufs=2))
        dpool = ctx.enter_context(tc.tile_pool(name="dpool", bufs=2))
        opool = ctx.enter_context(tc.tile_pool(name="opool", bufs=2))

        psA = ctx.enter_context(tc.tile_pool(name="psA", bufs=2, space="PSUM"))
        psB = ctx.enter_context(tc.tile_pool(name="psB", bufs=2, space="PSUM"))
        psC = ctx.enter_context(tc.tile_pool(name="psC", bufs=2, space="PSUM"))
        psN = ctx.enter_context(tc.tile_pool(name="psN", bufs=1, space="PSUM"))
        psO = ctx.enter_context(tc.tile_pool(name="psO", bufs=1, space="PSUM"))

        # ---- startup-critical loads: the first proj chain needs wts2 and
        # htsA(0) in kt order.  All initial DMAs can only issue after the
        # ~7us framework preamble and then compete for HBM, so the two
        # tensors are split into kt-ordered pieces interleaved on the sync
        # ring — the first matmul starts after ~400KB instead of ~3MB.
        wts = [None] * 4
        wts[2] = const.tile([128, 16, HD], BF16, name="wts2", tag="wts2")
        hts0A = hpool.tile([128, 8, SC_ATT], BF16, tag="htsA")
        for q in range(4):
            nc.sync.dma_start(out=wts[2][:, 4 * q:4 * q + 4, :],
                              in_=wts_d[2][:, 4 * q:4 * q + 4, :])
            nc.sync.dma_start(out=hts0A[:, 2 * q:2 * q + 2, :],
                              in_=hT[:, 0, 2 * q:2 * q + 2, :])

        # ---- small constants on the SWDGE queue ------------------------
        ident = const.tile([128, 128], BF16)
        make_identity(nc, ident)
        ones128 = const.tile([128, 128], F32R)
        nc.gpsimd.dma_start(out=ones128, in_=onesd[:, :])
        onesb = const.tile([128, 128], BF16)
        nc.gpsimd.dma_start(out=onesb, in_=onesbd[:, :])
        Rm = const.tile([128, 128], BF16)
        nc.gpsimd.dma_start(out=Rm, in_=rswapd[:, :])
        b4s = const.tile([128, 4], FP32)
        nc.gpsimd.dma_start(out=b4s, in_=b4[:, :])
        kc2s = const.tile([128, S], BF16)
        nc.gpsimd.dma_start(out=kc2s, in_=kc2[:, :])
        ks2s = const.tile([128, S], BF16)
        nc.gpsimd.dma_start(out=ks2s, in_=ks2[:, :])
        qc2s = const.tile([128, S], BF16)
        nc.gpsimd.dma_start(out=qc2s, in_=qc2[:, :])
        qs2s = const.tile([128, S], BF16)
        nc.gpsimd.dma_start(out=qs2s, in_=qs2[:, :])
        epsc = const.tile([128, 1], FP32)
        nc.vector.memset(epsc, EPS)
        # causal band masks generated on-device (keeps them out of the
        # startup HBM burst): mks[p, j, f] = 1 iff f >= p + 128*j
        mks = const.tile([128, 4, SC_ATT], BF16)
        nc.gpsimd.memset(mks, 1.0)
        for j in range(4):
            nc.gpsimd.affine_select(
                out=mks[:, j, :], in_=mks[:, j, :],
                compare_op=mybir.AluOpType.is_ge, fill=0.0,
                base=-128 * j, channel_multiplier=-1,
                pattern=[[1, SC_ATT]])

        rawq = [rawp.tile([128, S], BF16, tag=f"raw{i}", name=f"raw{i}")
                for i in range(3)]
        qhat = [atp.tile([128, S], BF16, tag=f"qh{i}", name=f"qh{i}")
                for i in range(2)]
        khat = atp.tile([128, S], BF16, tag="kh")
        attnT = [atp.tile([128, S], BF16, tag=f"attnT{h}", name=f"attnT{h}")
                 for h in range(2)]
        vsb = vpool.tile([128, 16, HD], BF16, tag="vsb")
        heads = [
            (rawq[2], khat, kc2s, ks2s),
            (rawq[0], qhat[0], qc2s, qs2s),
            (rawq[1], qhat[1], qc2s, qs2s),
        ]
        raw3s = {}

        # ================================================================
        # Software-pipelined emission: stage lag guarantees every
        # instruction's inputs are a full pipeline iteration old, so no
        # engine stream ever blocks at a phase boundary.
        #   iter sc: proj(sc) | rope+V(sc-1) | attention(sc-2) | oproj(sc-3)
        # ================================================================
        for it in range(NCH + 3):
            # ---- stage 1: qkv projection ------------------------------
            if it < NCH:
                sc = it
                sl = bass.ts(sc, SC_ATT)
                if sc == 0:
                    htsA = hts0A  # piece-loaded at the head of the sync ring
                else:
                    htsA = hpool.tile([128, 8, SC_ATT], BF16, tag="htsA")
                    nc.sync.dma_start(out=htsA, in_=hT[:, sc, 0:8, :])
                htsB = hpool.tile([128, 8, SC_ATT], BF16, tag="htsB")
                nc.scalar.dma_start(out=htsB, in_=hT[:, sc, 8:16, :])
                if sc == 0:
                    # remaining weights follow htsB on the scalar ring;
                    # woT rides the SWDGE queue (needed latest)
                    for oc in (0, 1, 3):
                        wt = const.tile([128, 16, HD], BF16,
                                        name=f"wts{oc}", tag=f"wts{oc}")
                        nc.scalar.dma_start(out=wt, in_=wts_d[oc][:, :, :])
                        wts[oc] = wt
                    woTs = const.tile([128, 2, HID], BF16)
                    nc.gpsimd.dma_start(out=woTs, in_=woT[:, :, :])
                for oc in (2, 0, 1, 3):
                    ps = psA.tile([128, SC_ATT], FP32, tag="mm")
                    for kt in range(16):
                        src_h = htsA if kt < 8 else htsB
                        nc.tensor.matmul(
                            ps, lhsT=wts[oc][:, kt, :],
                            rhs=src_h[:, kt % 8, :],
                            start=(kt == 0), stop=(kt == 15))
                    if oc == 3:
                        raw3 = wpool.tile([128, SC_ATT], BF16, tag="raw3",
                                          bufs=2)
                        nc.vector.tensor_scalar_add(raw3, ps,
                                                    b4s[:, oc:oc + 1])
                        raw3s[sc] = raw3
                    else:
                        nc.vector.tensor_scalar_add(
                            rawq[oc][:, sl], ps, b4s[:, oc:oc + 1])

            # ---- stage 2: rmsnorm + rope + V tiles --------------------
            if 1 <= it <= NCH:
                sc = it - 1
                sl = bass.ts(sc, SC_ATT)
                for raw, dst, wc2_, ws2_ in heads:
                    sq = wpool.tile([128, SC_ATT], BF16, tag="sq", bufs=2)
                    nc.vector.tensor_mul(sq, raw[:, sl], raw[:, sl])
                    ssum = psN.tile([128, SC_ATT], FP32, tag="norm")
                    nc.tensor.matmul(ssum, lhsT=onesb, rhs=sq,
                                     start=True, stop=True)
                    lnb = wpool.tile([128, SC_ATT], BF16, tag="lnb", bufs=2)
                    nc.scalar.activation(lnb, ssum, AF.Ln,
                                         scale=1.0 / HD, bias=epsc)
                    nc.scalar.activation(lnb, lnb, AF.Exp, scale=-0.5)
                    rtp = psN.tile([128, SC_ATT], FP32, tag="norm")
                    nc.tensor.matmul(rtp, lhsT=Rm, rhs=raw[:, sl],
                                     start=True, stop=True)
                    t1 = wpool.tile([128, SC_ATT], BF16, tag="tt", bufs=3)
                    nc.vector.tensor_mul(t1, raw[:, sl], wc2_[:, sl])
                    t2 = wpool.tile([128, SC_ATT], BF16, tag="tt", bufs=3)
                    nc.vector.tensor_mul(t2, rtp, ws2_[:, sl])
                    t3 = wpool.tile([128, SC_ATT], BF16, tag="tt", bufs=3)
                    nc.vector.tensor_add(t3, t1, t2)
                    nc.vector.tensor_mul(dst[:, sl], t3, lnb)
                raw3 = raw3s.pop(sc)
                for j in range(4):
                    tt = 4 * sc + j
                    vps = psN.tile([128, 128], BF16, tag="norm")
                    nc.tensor.transpose(vps,
                                        raw3[:, bass.ts(j, 128)], ident)
                    nc.vector.tensor_copy(vsb[:, tt, :], vps)

            # ---- stage 3: attention, both heads -----------------------
            # The last query chunk is processed in two 256-wide halves:
            # the first half skips the top key tiles entirely (less exp
            # work) and its o-projection overlaps the second half's
            # attention, shortening the serial tail of the kernel.
            if 2 <= it <= NCH + 1:
                sc = it - 2
                if sc < NCH - 1:
                    subs = [(sc * SC_ATT, SC_ATT)]
                else:
                    subs = [(sc * SC_ATT, SC_ATT // 2),
                            (sc * SC_ATT + SC_ATT // 2, SC_ATT // 2)]
                for q0, qn in subs:
                    for h in range(2):
                        ntt = (q0 + qn) // 128
                        outps = psO.tile([128, qn], FP32, tag="attnout")
                        dacc = dpool.tile([128, qn], F32R, tag="dacc")
                        t0 = 0
                        paired = qn * 2 * 4 <= 2048  # 2 tiles fit one bank
                        while t0 < ntt:
                            gn = min(4, ntt - t0)
                            pg = ppool.tile([128, 4, qn], BF16, tag="pt")
                            if paired:
                                # two key tiles share one PSUM bank and a
                                # single exp ACTIVATE (halves ACT op count)
                                for jp in range(0, gn, 2):
                                    scp2 = psC.tile([128, 2, qn], FP32,
                                                    tag="score")
                                    for u in range(2):
                                        tt = t0 + jp + u
                                        nc.tensor.matmul(
                                            scp2[:, u, :],
                                            lhsT=khat[:, bass.ts(tt, 128)],
                                            rhs=qhat[h][:, bass.ds(q0, qn)],
                                            start=True, stop=True)
                                    nc.scalar.activation(
                                        pg[:, jp:jp + 2, :], scp2,
                                        AF.Exp, scale=SCALE)
                                    for u in range(2):
                                        tt = t0 + jp + u
                                        j = jp + u
                                        jd = tt - q0 // 128
                                        if jd >= 0:
                                            nc.vector.tensor_mul(
                                                pg[:, j, :], pg[:, j, :],
                                                mks[:, jd, 0:qn])
                                        nc.tensor.matmul(
                                            outps, lhsT=vsb[:, tt, :],
                                            rhs=pg[:, j, :],
                                            start=(tt == 0),
                                            stop=(tt == ntt - 1))
                            else:
                                for j in range(gn):
                                    tt = t0 + j
                                    scp = psC.tile([128, qn], FP32,
                                                   tag="score")
                                    nc.tensor.matmul(
                                        scp, lhsT=khat[:, bass.ts(tt, 128)],
                                        rhs=qhat[h][:, bass.ds(q0, qn)],
                                        start=True, stop=True)
                                    nc.scalar.activation(pg[:, j, :], scp,
                                                         AF.Exp,
                                                         scale=SCALE)
                                    jd = tt - q0 // 128
                                    if jd >= 0:  # diagonal: zero t > s
                                        nc.vector.tensor_mul(
                                            pg[:, j, :], pg[:, j, :],
                                            mks[:, jd, 0:qn])
                                    nc.tensor.matmul(outps,
                                                     lhsT=vsb[:, tt, :],
                                                     rhs=pg[:, j, :],
                                                     start=(tt == 0),
                                                     stop=(tt == ntt - 1))
                            if gn == 4:
                                ga = ppool.tile([128, qn], BF16, tag="ga",
                                                bufs=2)
                                nc.vector.tensor_add(ga, pg[:, 0, :],
                                                     pg[:, 1, :])
                                gb = ppool.tile([128, qn], BF16, tag="gb",
                                                bufs=1)
                                nc.vector.tensor_add(gb, pg[:, 2, :],
                                                     pg[:, 3, :])
                                if t0 == 0:
                                    nc.vector.tensor_add(dacc, ga, gb)
                                else:
                                    gc = ppool.tile([128, qn], BF16,
                                                    tag="ga", bufs=2)
                                    nc.vector.tensor_add(gc, ga, gb)
                                    nc.vector.tensor_add(dacc, dacc, gc)
                            else:  # trailing pair (only in split chunks)
                                ga = ppool.tile([128, qn], BF16, tag="ga",
                                                bufs=2)
                                nc.vector.tensor_add(ga, pg[:, 0, :],
                                                     pg[:, 1, :])
                                nc.vector.tensor_add(dacc, dacc, ga)
                            t0 += gn
                        drep = psN.tile([128, qn], FP32, tag="norm")
                        nc.tensor.matmul(drep, lhsT=ones128, rhs=dacc,
                                         start=True, stop=True)
                        drec = wpool.tile([128, qn], FP32, tag="drec",
                                          bufs=2)
                        nc.vector.reciprocal_approx_fast(drec, drep)
                        nc.vector.tensor_mul(attnT[h][:, bass.ds(q0, qn)],
                                             outps, drec)

            # ---- stage 4: o projection --------------------------------
            if it >= 3:
                sc = it - 3
                # late chunks run with no proj/attention stages left, so
                # their oproj can also cycle through the idle psA (and,
                # for the final chunk, psC) banks — deeper PSUM pipeline
                # keeps the PE from stalling on PSUM->SBUF drains.
                if sc == NCH - 1:
                    opsum = [(psB, "omm"), (psA, "mm"), (psC, "score")]
                elif sc == NCH - 2:
                    opsum = [(psB, "omm"), (psA, "mm")]
                else:
                    opsum = [(psB, "omm")]
                pi = 0
                for st in range(4 * sc, 4 * sc + 4):
                    # whole output row block [128, HID] is staged in one
                    # SBUF tile and stored with a single large DMA
                    osb = opool.tile([128, HID], BF16, tag="osb")
                    for jp in range(HID // SC_ATT // 2):  # jc pairs
                        pool_, tag_ = opsum[pi % len(opsum)]
                        pi += 1
                        opsa = pool_.tile([128, SC_ATT], FP32, tag=tag_)
                        opsb = pool_.tile([128, SC_ATT], FP32, tag=tag_)
                        # jc pair shares each head's lhsT (one LDWEIGHTS
                        # per head instead of per matmul)
                        for h in range(2):
                            for ops, jc in ((opsa, 2 * jp),
                                            (opsb, 2 * jp + 1)):
                                nc.tensor.matmul(
                                    ops,
                                    lhsT=attnT[h][:, bass.ts(st, 128)],
                                    rhs=woTs[:, h, bass.ts(jc, SC_ATT)],
                                    start=(h == 0), stop=(h == 1))
                        nc.vector.tensor_copy(
                            osb[:, bass.ts(2 * jp, SC_ATT)], opsa)
                        nc.scalar.copy(
                            osb[:, bass.ts(2 * jp + 1, SC_ATT)], opsb)
                    if sc == NCH - 1 and st == 4 * sc + 3:
                        # very last row block: halves on both rings so the
                        # final store drain is not serialized on one ring
                        nc.sync.dma_start(
                            out=outp[bass.ts(st, 128), 0:HID // 2],
                            in_=osb[:, 0:HID // 2])
                        nc.scalar.dma_start(
                            out=outp[bass.ts(st, 128), HID // 2:HID],
                            in_=osb[:, HID // 2:HID])
                    else:
                        eng = nc.sync if st % 2 == 0 else nc.scalar
                        eng.dma_start(out=outp[bass.ts(st, 128), :],
                                      in_=osb)

    nc.compile()
    return nc


def _prep_inputs(hidden_states, cos, sin, wqkv, bqkv, wo, q_norm_w, k_norm_w):
    """Host-side layout prep + per-core sharding. All device tensors are
    pre-swizzled so every DMA has long contiguous per-partition runs."""
    import ml_dtypes
    bf16 = ml_dtypes.bfloat16
    f32 = np.float32
    hTn = np.ascontiguousarray(hidden_states.reshape(S, HID).T).astype(bf16)
    hTh = np.ascontiguousarray(
        hTn.reshape(16, 128, S // SC_ATT, SC_ATT).transpose(1, 2, 0, 3)
    )  # [p, sc, kt, s] — chunk-major so chunk loads are contiguous
    cosT = cos.T.astype(f32)  # [64, S]
    sinT = sin.T.astype(f32)
    cs2 = np.concatenate([cosT, cosT], axis=0)  # [128, S]
    ss2 = np.concatenate([sinT, sinT], axis=0)
    qwv = q_norm_w.astype(f32).reshape(128, 1)
    kwv = k_norm_w.astype(f32).reshape(128, 1)
    qwsv = np.concatenate([q_norm_w[64:], q_norm_w[:64]]).astype(
        f32).reshape(128, 1)
    kwsv = np.concatenate([k_norm_w[64:], k_norm_w[:64]]).astype(
        f32).reshape(128, 1)
    qc2 = np.ascontiguousarray(qwv * cs2).astype(bf16)
    qs2 = np.ascontiguousarray(qwsv * ss2).astype(bf16)
    kc2 = np.ascontiguousarray(kwv * cs2).astype(bf16)
    ks2 = np.ascontiguousarray(kwsv * ss2).astype(bf16)
    ones_np = np.ones((128, 128), dtype=f32)
    onesb_np = np.ones((128, 128), dtype=bf16)
    rt = np.zeros((128, 128), dtype=f32)
    rt[np.arange(64) + 64, np.arange(64)] = -1.0   # R^T[d+64, d] = -1
    rt[np.arange(64), np.arange(64) + 64] = 1.0    # R^T[d-64, d] = +1
    rt = rt.astype(bf16)

    in_maps = []
    for c in range(N_CORES):
        kvh = c // 2
        rows = list(range(2 * c * HD, (2 * c + 2) * HD))          # q0, q1
        rows += list(range(NH * HD + kvh * HD, NH * HD + (kvh + 1) * HD))  # k
        rows += list(range((NH + NKV) * HD + kvh * HD,
                           (NH + NKV) * HD + (kvh + 1) * HD))      # v
        w_c = wqkv[rows]                       # [512, HID]
        wTc = np.ascontiguousarray(w_c.T).astype(f32)   # [HID, 512]
        wTk = wTc.reshape(16, 128, 512)
        b_c = bqkv[rows].astype(f32)           # [512]
        b4c = np.ascontiguousarray(b_c.reshape(4, 128).T)  # [128, 4]
        woc = wo[:, 2 * c * HD:(2 * c + 2) * HD]  # [HID, 256]
        woTc = np.ascontiguousarray(woc.T).astype(f32)  # [256, HID]
        woTh = np.ascontiguousarray(
            woTc.reshape(2, 128, HID).transpose(1, 0, 2)).astype(bf16)
        im = {
            "hT": hTh, "b4": b4c, "woT": woTh,
            "qc2": qc2, "qs2": qs2, "kc2": kc2, "ks2": ks2,
            "ones": ones_np, "onesb": onesb_np, "rswap": rt,
        }
        for oc in range(4):
            im[f"wT{oc}"] = np.ascontiguousarray(
                wTk[:, :, oc * 128:(oc + 1) * 128].transpose(1, 0, 2)
            ).astype(bf16)
        in_maps.append(im)
    return in_maps


_NC_CACHE = {}


def kernel(hidden_states, cos, sin, k_cache, v_cache, mask,
           wqkv, bqkv, wo, bo, q_norm_w, k_norm_w, kv_write_indices,
           trace=False):
    hidden_states = np.asarray(hidden_states, dtype=np.float32)
    in_maps = _prep_inputs(
        np.asarray(hidden_states), np.asarray(cos), np.asarray(sin),
        np.asarray(wqkv), np.asarray(bqkv), np.asarray(wo),
        np.asarray(q_norm_w), np.asarray(k_norm_w))

    if "nc" not in _NC_CACHE:
        _NC_CACHE["nc"] = build_nc()
    nc = _NC_CACHE["nc"]

    res = run_bass_kernel_spmd(nc, in_maps, core_ids=list(range(N_CORES)),
                               trace=trace)
    out = np.zeros((S, HID), np.float32)
    for rmap in res.results:
        out += np.asarray(rmap["outp"], dtype=np.float32)
    out += np.asarray(bo, dtype=np.float32)[None, :]
    if trace:
        kernel.last_results = res
    return out.reshape(1, S, HID)


# revision 35
# speedup vs baseline: 1.0942x; 1.0064x over previous
"""Trainium2 Bass kernel for nn_Attention_3736621547687.

B=1, S=2048, HID=2048, NH=16, NKV=4, HD=128 attention block:
qkv proj -> per-head RMSNorm(q,k) -> RoPE -> causal GQA attention -> o proj.

Sharding: tensor-parallel over heads across 8 cores. Core c owns q heads
{2c, 2c+1} and kv head c//2 (replicated across the pair of cores sharing it).
Each core computes a partial o-projection output; the host sums the 8
partials (Megatron-style row-parallel reduce) and adds the output bias.

Device-side layout trick: everything is computed in "transposed" orientation
(feature dim on partitions, sequence on the free dim) so that no on-chip
transposes of activations are needed:
  - host supplies hidden^T, wqkv_c^T, wo_c^T, cos/sin tiled to [128, S]
  - qkv proj emits q^T/k^T/v^T directly
  - scores are computed as scores^T [keys, queries]; softmax denominators are
    partition-dim sums obtained with an all-ones [128,128] matmul that also
    replicates the result across partitions (giving the broadcast for free)
  - exp() is fused with the 1/sqrt(qpa) scale on the scalar engine; causal
    masking = multiplying exp values by a 0/1 band mask on the vector engine
    (identical to the reference's additive -1e9 mask in fp32)
  - softmax max-subtraction is skipped: scores are ~N(0,1) after RMSNorm so
    exp() cannot overflow; mathematically identical to the reference.
All activations/weights are bf16 (fast weight load, 2x DVE modes, half the
HBM traffic); accumulations stay in fp32 PSUM.
"""

import numpy as np
from contextlib import ExitStack

import concourse.bass as bass
import concourse.bacc as bacc
import concourse.mybir as mybir
import concourse.tile as tile
from concourse.masks import make_identity
from concourse.bass_utils import run_bass_kernel_spmd

S = 2048
HID = 2048
NH = 16
NKV = 4
HD = 128
G = NH // NKV
SCALE = float(128.0 ** -0.5)  # query_pre_attn_scalar = 128
EPS = 1e-6

FP32 = mybir.dt.float32
F32R = mybir.dt.float32r
BF16 = mybir.dt.bfloat16
MULT = mybir.AluOpType.mult
AF = mybir.ActivationFunctionType

N_CORES = 8
SC_ATT = 512    # attention/oproj moving-dim chunk


def _patch_act_tables():
    """Force Ln and Exp onto the single combined activation-table set so the
    scalar engine never reloads tables when rms-norm and softmax interleave.
    Set ids must keep their positions, so competing sets are emptied rather
    than removed."""
    import concourse.hw_specs as hw_specs
    import concourse.bacc as bacc_mod
    orig = hw_specs.get_activation_tables

    def patched(module_arch):
        t = orig(module_arch)
        for name in ("exp_and_others", "natural_log", "exp_and_friends"):
            if name in t and "natural_log_exp_and_others" in t:
                t[name] = set()
        return t

    bacc_mod.get_activation_tables = patched


def build_nc():
    _patch_act_tables()
    nc = bacc.Bacc()

    NCH = S // SC_ATT
    hT = nc.dram_tensor("hT", [128, NCH, 16, SC_ATT], BF16,
                        kind="ExternalInput")
    wts_d = [nc.dram_tensor(f"wT{i}", [128, 16, HD], BF16, kind="ExternalInput")
             for i in range(4)]
    b4 = nc.dram_tensor("b4", [128, 4], FP32, kind="ExternalInput")
    woT = nc.dram_tensor("woT", [128, 2, HID], BF16, kind="ExternalInput")
    # rope tables with the rms-norm weight folded in per partition:
    # qc2 = qw*[cos;cos], qs2 = swap(qw)*[sin;sin], likewise for k
    qc2 = nc.dram_tensor("qc2", [128, S], BF16, kind="ExternalInput")
    qs2 = nc.dram_tensor("qs2", [128, S], BF16, kind="ExternalInput")
    kc2 = nc.dram_tensor("kc2", [128, S], BF16, kind="ExternalInput")
    ks2 = nc.dram_tensor("ks2", [128, S], BF16, kind="ExternalInput")
    onesd = nc.dram_tensor("ones", [128, 128], F32R, kind="ExternalInput")
    onesbd = nc.dram_tensor("onesb", [128, 128], BF16, kind="ExternalInput")
    rswapd = nc.dram_tensor("rswap", [128, 128], BF16, kind="ExternalInput")
    outp = nc.dram_tensor("outp", [S, HID], BF16, kind="ExternalOutput")

    with ExitStack() as ctx:
        tc = ctx.enter_context(tile.TileContext(nc))

        const = ctx.enter_context(tc.tile_pool(name="const", bufs=1))
        hpool = ctx.enter_context(tc.tile_pool(name="hpool", bufs=2))
        rawp = ctx.enter_context(tc.tile_pool(name="rawp", bufs=1))
        atp = ctx.enter_context(tc.tile_pool(name="atp", bufs=1))
        wpool = ctx.enter_context(tc.tile_pool(name="wpool", bufs=3))
        vpool = ctx.enter_context(tc.tile_pool(name="vpool", bufs=1))
        ppool = ctx.enter_context(tc.tile_pool(name="ppool", bufs=2))
        dpool = ctx.enter_context(tc.tile_pool(name="dpool", bufs=2))
        opool = ctx.enter_context(tc.tile_pool(name="opool", bufs=2))

        psA = ctx.enter_context(tc.tile_pool(name="psA", bufs=2, space="PSUM"))
        psB = ctx.enter_context(tc.tile_pool(name="psB", bufs=2, space="PSUM"))
        psC = ctx.enter_context(tc.tile_pool(name="psC", bufs=2, space="PSUM"))
        psN = ctx.enter_context(tc.tile_pool(name="psN", bufs=1, space="PSUM"))
        psO = ctx.enter_context(tc.tile_pool(name="psO", bufs=1, space="PSUM"))

        # ---- weight loads ride the scalar HWDGE ring so they overlap the
        # chunk-0 hidden-state load on the sync ring (k-head slice first).
        # The startup ramp is HBM-burst-bound: all initial DMAs issue after
        # the ~7us framework preamble and share bandwidth with the other 7
        # cores, so the first matmul lands ~13-15us in regardless of order.
        wts = [None] * 4
        wts[2] = const.tile([128, 16, HD], BF16, name="wts2", tag="wts2")
        nc.scalar.dma_start(out=wts[2], in_=wts_d[2][:, :, :])

        # ---- small constants on the SWDGE queue ------------------------
        ident = const.tile([128, 128], BF16)
        make_identity(nc, ident)
        ones128 = const.tile([128, 128], F32R)
        nc.gpsimd.dma_start(out=ones128, in_=onesd[:, :])
        onesb = const.tile([128, 128], BF16)
        nc.gpsimd.dma_start(out=onesb, in_=onesbd[:, :])
        Rm = const.tile([128, 128], BF16)
        nc.gpsimd.dma_start(out=Rm, in_=rswapd[:, :])
        b4s = const.tile([128, 4], FP32)
        nc.gpsimd.dma_start(out=b4s, in_=b4[:, :])
        kc2s = const.tile([128, S], BF16)
        nc.gpsimd.dma_start(out=kc2s, in_=kc2[:, :])
        ks2s = const.tile([128, S], BF16)
        nc.gpsimd.dma_start(out=ks2s, in_=ks2[:, :])
        qc2s = const.tile([128, S], BF16)
        nc.gpsimd.dma_start(out=qc2s, in_=qc2[:, :])
        qs2s = const.tile([128, S], BF16)
        nc.gpsimd.dma_start(out=qs2s, in_=qs2[:, :])
        epsc = const.tile([128, 1], FP32)
        nc.vector.memset(epsc, EPS)
        # causal band masks generated on-device (keeps them out of the
        # startup HBM burst): mks[p, j, f] = 1 iff f >= p + 128*j
        mks = const.tile([128, 4, SC_ATT], BF16)
        nc.gpsimd.memset(mks, 1.0)
        for j in range(4):
            nc.gpsimd.affine_select(
                out=mks[:, j, :], in_=mks[:, j, :],
                compare_op=mybir.AluOpType.is_ge, fill=0.0,
                base=-128 * j, channel_multiplier=-1,
                pattern=[[1, SC_ATT]])

        rawq = [rawp.tile([128, S], BF16, tag=f"raw{i}", name=f"raw{i}")
                for i in range(3)]
        qhat = [atp.tile([128, S], BF16, tag=f"qh{i}", name=f"qh{i}")
                for i in range(2)]
        khat = atp.tile([128, S], BF16, tag="kh")
        attnT = [atp.tile([128, S], BF16, tag=f"attnT{h}", name=f"attnT{h}")
                 for h in range(2)]
        vsb = vpool.tile([128, 16, HD], BF16, tag="vsb")
        heads = [
            (rawq[2], khat, kc2s, ks2s),
            (rawq[0], qhat[0], qc2s, qs2s),
            (rawq[1], qhat[1], qc2s, qs2s),
        ]
        raw3s = {}

        # ================================================================
        # Software-pipelined emission: stage lag guarantees every
        # instruction's inputs are a full pipeline iteration old, so no
        # engine stream ever blocks at a phase boundary.
        #   iter sc: proj(sc) | rope+V(sc-1) | attention(sc-2) | oproj(sc-3)
        # ================================================================
        for it in range(NCH + 3):
            # ---- stage 1: qkv projection ------------------------------
            if it < NCH:
                sc = it
                sl = bass.ts(sc, SC_ATT)
                if sc == 0:
                    htsA = hts0A  # piece-loaded at the head of the sync ring
                else:
                    htsA = hpool.tile([128, 8, SC_ATT], BF16, tag="htsA")
                    nc.sync.dma_start(out=htsA, in_=hT[:, sc, 0:8, :])
                htsB = hpool.tile([128, 8, SC_ATT], BF16, tag="htsB")
                nc.scalar.dma_start(out=htsB, in_=hT[:, sc, 8:16, :])
                if sc == 0:
                    # remaining weights follow htsB on the scalar ring;
                    # woT rides the SWDGE queue (needed latest)
                    for oc in (0, 1, 3):
                        wt = const.tile([128, 16, HD], BF16,
                                        name=f"wts{oc}", tag=f"wts{oc}")
                        nc.scalar.dma_start(out=wt, in_=wts_d[oc][:, :, :])
                        wts[oc] = wt
                    woTs = const.tile([128, 2, HID], BF16)
                    nc.gpsimd.dma_start(out=woTs, in_=woT[:, :, :])
                for oc in (2, 0, 1, 3):
                    ps = psA.tile([128, SC_ATT], FP32, tag="mm")
                    for kt in range(16):
                        src_h = htsA if kt < 8 else htsB
                        nc.tensor.matmul(
                            ps, lhsT=wts[oc][:, kt, :],
                            rhs=src_h[:, kt % 8, :],
                            start=(kt == 0), stop=(kt == 15))
                    if oc == 3:
                        raw3 = wpool.tile([128, SC_ATT], BF16, tag="raw3",
                                          bufs=2)
                        nc.vector.tensor_scalar_add(raw3, ps,
                                                    b4s[:, oc:oc + 1])
                        raw3s[sc] = raw3
                    else:
                        nc.vector.tensor_scalar_add(
                            rawq[oc][:, sl], ps, b4s[:, oc:oc + 1])

            # ---- stage 2: rmsnorm + rope + V tiles --------------------
            if 1 <= it <= NCH:
                sc = it - 1
                sl = bass.ts(sc, SC_ATT)
                for raw, dst, wc2_, ws2_ in heads:
                    sq = wpool.tile([128, SC_ATT], BF16, tag="sq", bufs=2)
                    nc.vector.tensor_mul(sq, raw[:, sl], raw[:, sl])
                    ssum = psN.tile([128, SC_ATT], FP32, tag="norm")
                    nc.tensor.matmul(ssum, lhsT=onesb, rhs=sq,
                                     start=True, stop=True)
                    lnb = wpool.tile([128, SC_ATT], BF16, tag="lnb", bufs=2)
                    nc.scalar.activation(lnb, ssum, AF.Ln,
                                         scale=1.0 / HD, bias=epsc)
                    nc.scalar.activation(lnb, lnb, AF.Exp, scale=-0.5)
                    rtp = psN.tile([128, SC_ATT], FP32, tag="norm")
                    nc.tensor.matmul(rtp, lhsT=Rm, rhs=raw[:, sl],
                                     start=True, stop=True)
                    t1 = wpool.tile([128, SC_ATT], BF16, tag="tt", bufs=3)
                    nc.vector.tensor_mul(t1, raw[:, sl], wc2_[:, sl])
                    t2 = wpool.tile([128, SC_ATT], BF16, tag="tt", bufs=3)
                    nc.vector.tensor_mul(t2, rtp, ws2_[:, sl])
                    t3 = wpool.tile([128, SC_ATT], BF16, tag="tt", bufs=3)
                    nc.vector.tensor_add(t3, t1, t2)
                    nc.vector.tensor_mul(dst[:, sl], t3, lnb)
                raw3 = raw3s.pop(sc)
                for j in range(4):
                    tt = 4 * sc + j
                    vps = psN.tile([128, 128], BF16, tag="norm")
                    nc.tensor.transpose(vps,
                                        raw3[:, bass.ts(j, 128)], ident)
                    nc.vector.tensor_copy(vsb[:, tt, :], vps)

            # ---- stage 3: attention, both heads -----------------------
            # The last query chunk is processed in two 256-wide halves:
            # the first half skips the top key tiles entirely (less exp
            # work) and its o-projection overlaps the second half's
            # attention, shortening the serial tail of the kernel.
            if 2 <= it <= NCH + 1:
                sc = it - 2
                if sc < NCH - 1:
                    subs = [(sc * SC_ATT, SC_ATT)]
                else:
                    subs = [(sc * SC_ATT, SC_ATT // 2),
                            (sc * SC_ATT + SC_ATT // 2, SC_ATT // 2)]
                for q0, qn in subs:
                    for h in range(2):
                        ntt = (q0 + qn) // 128
                        outps = psO.tile([128, qn], FP32, tag="attnout")
                        dacc = dpool.tile([128, qn], F32R, tag="dacc")
                        t0 = 0
                        paired = qn * 2 * 4 <= 2048  # 2 tiles fit one bank
                        while t0 < ntt:
                            gn = min(4, ntt - t0)
                            pg = ppool.tile([128, 4, qn], BF16, tag="pt")
                            if paired:
                                # two key tiles share one PSUM bank and a
                                # single exp ACTIVATE (halves ACT op count)
                                for jp in range(0, gn, 2):
                                    scp2 = psC.tile([128, 2, qn], FP32,
                                                    tag="score")
                                    for u in range(2):
                                        tt = t0 + jp + u
                                        nc.tensor.matmul(
                                            scp2[:, u, :],
                                            lhsT=khat[:, bass.ts(tt, 128)],
                                            rhs=qhat[h][:, bass.ds(q0, qn)],
                                            start=True, stop=True)
                                    nc.scalar.activation(
                                        pg[:, jp:jp + 2, :], scp2,
                                        AF.Exp, scale=SCALE)
                                    for u in range(2):
                                        tt = t0 + jp + u
                                        j = jp + u
                                        jd = tt - q0 // 128
                                        if jd >= 0:
                                            nc.vector.tensor_mul(
                                                pg[:, j, :], pg[:, j, :],
                                                mks[:, jd, 0:qn])
                                        nc.tensor.matmul(
                                            outps, lhsT=vsb[:, tt, :],
                                            rhs=pg[:, j, :],
                                            start=(tt == 0),
                                            stop=(tt == ntt - 1))
                            else:
                                for j in range(gn):
                                    tt = t0 + j
                                    scp = psC.tile([128, qn], FP32,
                                                   tag="score")
                                    nc.tensor.matmul(
                                        scp, lhsT=khat[:, bass.ts(tt, 128)],
                                        rhs=qhat[h][:, bass.ds(q0, qn)],
                                        start=True, stop=True)
                                    nc.scalar.activation(pg[:, j, :], scp,
                                                         AF.Exp,
                                                         scale=SCALE)
                                    jd = tt - q0 // 128
                                    if jd >= 0:  # diagonal: zero t > s
                                        nc.vector.tensor_mul(
                                            pg[:, j, :], pg[:, j, :],
                                            mks[:, jd, 0:qn])
                                    nc.tensor.matmul(outps,
                                                     lhsT=vsb[:, tt, :],
                                                     rhs=pg[:, j, :],
                                                     start=(tt == 0),
                                                     stop=(tt == ntt - 1))
                            if gn == 4:
                                ga = ppool.tile([128, qn], BF16, tag="ga",
                                                bufs=2)
                                nc.vector.tensor_add(ga, pg[:, 0, :],
                                                     pg[:, 1, :])
                                gb = ppool.tile([128, qn], BF16, tag="gb",
                                                bufs=1)
                                nc.vector.tensor_add(gb, pg[:, 2, :],
                                                     pg[:, 3, :])
                                if t0 == 0:
                                    nc.vector.tensor_add(dacc, ga, gb)
                                else:
                                    gc = ppool.tile([128, qn], BF16,
                                                    tag="ga", bufs=2)
                                    nc.vector.tensor_add(gc, ga, gb)
                                    nc.vector.tensor_add(dacc, dacc, gc)
                            else:  # trailing pair (only in split chunks)
                                ga = ppool.tile([128, qn], BF16, tag="ga",
                                                bufs=2)
                                nc.vector.tensor_add(ga, pg[:, 0, :],
                                                     pg[:, 1, :])
                                nc.vector.tensor_add(dacc, dacc, ga)
                            t0 += gn
                        drep = psN.tile([128, qn], FP32, tag="norm")
                        nc.tensor.matmul(drep, lhsT=ones128, rhs=dacc,
                                         start=True, stop=True)
                        drec = wpool.tile([128, qn], FP32, tag="drec",
                                          bufs=2)
                        nc.vector.reciprocal_approx_fast(drec, drep)
                        nc.vector.tensor_mul(attnT[h][:, bass.ds(q0, qn)],
                                             outps, drec)

            # ---- stage 4: o projection --------------------------------
            if it >= 3:
                sc = it - 3
                # late chunks run with no proj/attention stages left, so
                # their oproj can also cycle through the idle psA (and,
                # for the final chunk, psC) banks — deeper PSUM pipeline
                # keeps the PE from stalling on PSUM->SBUF drains.
                if sc == NCH - 1:
                    opsum = [(psB, "omm"), (psA, "mm"), (psC, "score")]
                elif sc == NCH - 2:
                    opsum = [(psB, "omm"), (psA, "mm")]
                else:
                    opsum = [(psB, "omm")]
                pi = 0
                for st in range(4 * sc, 4 * sc + 4):
                    # whole output row block [128, HID] is staged in one
                    # SBUF tile and stored with a single large DMA
                    osb = opool.tile([128, HID], BF16, tag="osb")
                    for jp in range(HID // SC_ATT // 2):  # jc pairs
                        pool_, tag_ = opsum[pi % len(opsum)]
                        pi += 1
                        opsa = pool_.tile([128, SC_ATT], FP32, tag=tag_)
                        opsb = pool_.tile([128, SC_ATT], FP32, tag=tag_)
                        # jc pair shares each head's lhsT (one LDWEIGHTS
                        # per head instead of per matmul)
                        for h in range(2):
                            for ops, jc in ((opsa, 2 * jp),
                                            (opsb, 2 * jp + 1)):
                                nc.tensor.matmul(
                                    ops,
                                    lhsT=attnT[h][:, bass.ts(st, 128)],
                                    rhs=woTs[:, h, bass.ts(jc, SC_ATT)],
                                    start=(h == 0), stop=(h == 1))
                        nc.vector.tensor_copy(
                            osb[:, bass.ts(2 * jp, SC_ATT)], opsa)
                        nc.scalar.copy(
                            osb[:, bass.ts(2 * jp + 1, SC_ATT)], opsb)
                    if sc == NCH - 1 and st == 4 * sc + 3:
                        # very last row block: halves on both rings so the
                        # final store drain is not serialized on one ring
                        nc.sync.dma_start(
                            out=outp[bass.ts(st, 128), 0:HID // 2],
                            in_=osb[:, 0:HID // 2])
                        nc.scalar.dma_start(
                            out=outp[bass.ts(st, 128), HID // 2:HID],
                            in_=osb[:, HID // 2:HID])
                    else:
                        eng = nc.sync if st % 2 == 0 else nc.scalar
                        eng.dma_start(out=outp[bass.ts(st, 128), :],
                                      in_=osb)

    nc.compile()
    return nc


def _prep_inputs(hidden_states, cos, sin, wqkv, bqkv, wo, q_norm_w, k_norm_w):
    """Host-side layout prep + per-core sharding. All device tensors are
    pre-swizzled so every DMA has long contiguous per-partition runs."""
    import ml_dtypes
    bf16 = ml_dtypes.bfloat16
    f32 = np.float32
    hTn = np.ascontiguousarray(hidden_states.reshape(S, HID).T).astype(bf16)
    hTh = np.ascontiguousarray(
        hTn.reshape(16, 128, S // SC_ATT, SC_ATT).transpose(1, 2, 0, 3)
    )  # [p, sc, kt, s] — chunk-major so chunk loads are contiguous
    cosT = cos.T.astype(f32)  # [64, S]
    sinT = sin.T.astype(f32)
    cs2 = np.concatenate([cosT, cosT], axis=0)  # [128, S]
    ss2 = np.concatenate([sinT, sinT], axis=0)
    qwv = q_norm_w.astype(f32).reshape(128, 1)
    kwv = k_norm_w.astype(f32).reshape(128, 1)
    qwsv = np.concatenate([q_norm_w[64:], q_norm_w[:64]]).astype(
        f32).reshape(128, 1)
    kwsv = np.concatenate([k_norm_w[64:], k_norm_w[:64]]).astype(
        f32).reshape(128, 1)
    qc2 = np.ascontiguousarray(qwv * cs2).astype(bf16)
    qs2 = np.ascontiguousarray(qwsv * ss2).astype(bf16)
    kc2 = np.ascontiguousarray(kwv * cs2).astype(bf16)
    ks2 = np.ascontiguousarray(kwsv * ss2).astype(bf16)
    ones_np = np.ones((128, 128), dtype=f32)
    onesb_np = np.ones((128, 128), dtype=bf16)
    rt = np.zeros((128, 128), dtype=f32)
    rt[np.arange(64) + 64, np.arange(64)] = -1.0   # R^T[d+64, d] = -1
    rt[np.arange(64), np.arange(64) + 64] = 1.0    # R^T[d-64, d] = +1
    rt = rt.astype(bf16)

    in_maps = []
    for c in range(N_CORES):
        kvh = c // 2
        rows = list(range(2 * c * HD, (2 * c + 2) * HD))          # q0, q1
        rows += list(range(NH * HD + kvh * HD, NH * HD + (kvh + 1) * HD))  # k
        rows += list(range((NH + NKV) * HD + kvh * HD,
                           (NH + NKV) * HD + (kvh + 1) * HD))      # v
        w_c = wqkv[rows]                       # [512, HID]
        wTc = np.ascontiguousarray(w_c.T).astype(f32)   # [HID, 512]
        wTk = wTc.reshape(16, 128, 512)
        b_c = bqkv[rows].astype(f32)           # [512]
        b4c = np.ascontiguousarray(b_c.reshape(4, 128).T)  # [128, 4]
        woc = wo[:, 2 * c * HD:(2 * c + 2) * HD]  # [HID, 256]
        woTc = np.ascontiguousarray(woc.T).astype(f32)  # [256, HID]
        woTh = np.ascontiguousarray(
            woTc.reshape(2, 128, HID).transpose(1, 0, 2)).astype(bf16)
        im = {
            "hT": hTh, "b4": b4c, "woT": woTh,
            "qc2": qc2, "qs2": qs2, "kc2": kc2, "ks2": ks2,
            "ones": ones_np, "onesb": onesb_np, "rswap": rt,
        }
        for oc in range(4):
            im[f"wT{oc}"] = np.ascontiguousarray(
                wTk[:, :, oc * 128:(oc + 1) * 128].transpose(1, 0, 2)
            ).astype(bf16)
        in_maps.append(im)
    return in_maps


_NC_CACHE = {}


def kernel(hidden_states, cos, sin, k_cache, v_cache, mask,
           wqkv, bqkv, wo, bo, q_norm_w, k_norm_w, kv_write_indices,
           trace=False):
    hidden_states = np.asarray(hidden_states, dtype=np.float32)
    in_maps = _prep_inputs(
        np.asarray(hidden_states), np.asarray(cos), np.asarray(sin),
        np.asarray(wqkv), np.asarray(bqkv), np.asarray(wo),
        np.asarray(q_norm_w), np.asarray(k_norm_w))

    if "nc" not in _NC_CACHE:
        _NC_CACHE["nc"] = build_nc()
    nc = _NC_CACHE["nc"]

    res = run_bass_kernel_spmd(nc, in_maps, core_ids=list(range(N_CORES)),
                               trace=trace)
    out = np.zeros((S, HID), np.float32)
    for rmap in res.results:
        out += np.asarray(rmap["outp"], dtype=np.float32)
    out += np.asarray(bo, dtype=np.float32)[None, :]
    if trace:
        kernel.last_results = res
    return out.reshape(1, S, HID)


# revision 36
# speedup vs baseline: 1.2041x; 1.1004x over previous
"""Trainium2 Bass kernel for nn_Attention_3736621547687.

B=1, S=2048, HID=2048, NH=16, NKV=4, HD=128 attention block:
qkv proj -> per-head RMSNorm(q,k) -> RoPE -> causal GQA attention -> o proj.

Sharding: tensor-parallel over heads across 8 cores. Core c owns q heads
{2c, 2c+1} and kv head c//2 (replicated across the pair of cores sharing it).
Each core computes a partial o-projection output; the host sums the 8
partials (Megatron-style row-parallel reduce) and adds the output bias.

Device-side layout trick: everything is computed in "transposed" orientation
(feature dim on partitions, sequence on the free dim) so that no on-chip
transposes of activations are needed:
  - host supplies hidden^T, wqkv_c^T, wo_c^T, cos/sin tiled to [128, S]
  - qkv proj emits q^T/k^T/v^T directly
  - scores are computed as scores^T [keys, queries]; softmax denominators are
    partition-dim sums obtained with an all-ones [128,128] matmul that also
    replicates the result across partitions (giving the broadcast for free)
  - exp() is fused with the 1/sqrt(qpa) scale on the scalar engine; causal
    masking = multiplying exp values by a 0/1 band mask on the vector engine
    (identical to the reference's additive -1e9 mask in fp32)
  - softmax max-subtraction is skipped: scores are ~N(0,1) after RMSNorm so
    exp() cannot overflow; mathematically identical to the reference.
All activations/weights are bf16 (fast weight load, 2x DVE modes, half the
HBM traffic); accumulations stay in fp32 PSUM.
"""

import numpy as np
from contextlib import ExitStack

import concourse.bass as bass
import concourse.bacc as bacc
import concourse.mybir as mybir
import concourse.tile as tile
from concourse.masks import make_identity
from concourse.bass_utils import run_bass_kernel_spmd

S = 2048
HID = 2048
NH = 16
NKV = 4
HD = 128
G = NH // NKV
SCALE = float(128.0 ** -0.5)  # query_pre_attn_scalar = 128
EPS = 1e-6

FP32 = mybir.dt.float32
F32R = mybir.dt.float32r
BF16 = mybir.dt.bfloat16
MULT = mybir.AluOpType.mult
AF = mybir.ActivationFunctionType

N_CORES = 8
SC_ATT = 512    # attention/oproj moving-dim chunk


def _patch_act_tables():
    """Force Ln and Exp onto the single combined activation-table set so the
    scalar engine never reloads tables when rms-norm and softmax interleave.
    Set ids must keep their positions, so competing sets are emptied rather
    than removed."""
    import concourse.hw_specs as hw_specs
    import concourse.bacc as bacc_mod
    orig = hw_specs.get_activation_tables

    def patched(module_arch):
        t = orig(module_arch)
        for name in ("exp_and_others", "natural_log", "exp_and_friends"):
            if name in t and "natural_log_exp_and_others" in t:
                t[name] = set()
        return t

    bacc_mod.get_activation_tables = patched


def build_nc():
    _patch_act_tables()
    nc = bacc.Bacc()

    NCH = S // SC_ATT
    hT = nc.dram_tensor("hT", [128, NCH, 16, SC_ATT], BF16,
                        kind="ExternalInput")
    wts_d = [nc.dram_tensor(f"wT{i}", [128, 16, HD], BF16, kind="ExternalInput")
             for i in range(4)]
    b4 = nc.dram_tensor("b4", [128, 4], FP32, kind="ExternalInput")
    woT = nc.dram_tensor("woT", [128, 2, HID], BF16, kind="ExternalInput")
    # rope tables with the rms-norm weight folded in per partition:
    # qc2 = qw*[cos;cos], qs2 = swap(qw)*[sin;sin], likewise for k
    qc2 = nc.dram_tensor("qc2", [128, S], BF16, kind="ExternalInput")
    qs2 = nc.dram_tensor("qs2", [128, S], BF16, kind="ExternalInput")
    kc2 = nc.dram_tensor("kc2", [128, S], BF16, kind="ExternalInput")
    ks2 = nc.dram_tensor("ks2", [128, S], BF16, kind="ExternalInput")
    onesd = nc.dram_tensor("ones", [128, 128], F32R, kind="ExternalInput")
    onesbd = nc.dram_tensor("onesb", [128, 128], BF16, kind="ExternalInput")
    rswapd = nc.dram_tensor("rswap", [128, 128], BF16, kind="ExternalInput")
    outp = nc.dram_tensor("outp", [S, HID], BF16, kind="ExternalOutput")

    with ExitStack() as ctx:
        tc = ctx.enter_context(tile.TileContext(nc))

        const = ctx.enter_context(tc.tile_pool(name="const", bufs=1))
        hpool = ctx.enter_context(tc.tile_pool(name="hpool", bufs=2))
        rawp = ctx.enter_context(tc.tile_pool(name="rawp", bufs=1))
        atp = ctx.enter_context(tc.tile_pool(name="atp", bufs=1))
        wpool = ctx.enter_context(tc.tile_pool(name="wpool", bufs=3))
        vpool = ctx.enter_context(tc.tile_pool(name="vpool", bufs=1))
        ppool = ctx.enter_context(tc.tile_pool(name="ppool", bufs=2))
        dpool = ctx.enter_context(tc.tile_pool(name="dpool", bufs=2))
        opool = ctx.enter_context(tc.tile_pool(name="opool", bufs=2))

        psA = ctx.enter_context(tc.tile_pool(name="psA", bufs=2, space="PSUM"))
        psB = ctx.enter_context(tc.tile_pool(name="psB", bufs=2, space="PSUM"))
        psC = ctx.enter_context(tc.tile_pool(name="psC", bufs=2, space="PSUM"))
        psN = ctx.enter_context(tc.tile_pool(name="psN", bufs=1, space="PSUM"))
        psO = ctx.enter_context(tc.tile_pool(name="psO", bufs=1, space="PSUM"))

        # ---- weight loads ride the scalar HWDGE ring so they overlap the
        # chunk-0 hidden-state load on the sync ring (k-head slice first).
        # The startup ramp is HBM-burst-bound: all initial DMAs issue after
        # the ~7us framework preamble and share bandwidth with the other 7
        # cores, so the first matmul lands ~13-15us in regardless of order.
        wts = [None] * 4
        wts[2] = const.tile([128, 16, HD], BF16, name="wts2", tag="wts2")
        nc.scalar.dma_start(out=wts[2], in_=wts_d[2][:, :, :])

        # ---- small constants on the SWDGE queue ------------------------
        ident = const.tile([128, 128], BF16)
        make_identity(nc, ident)
        ones128 = const.tile([128, 128], F32R)
        nc.gpsimd.dma_start(out=ones128, in_=onesd[:, :])
        onesb = const.tile([128, 128], BF16)
        nc.gpsimd.dma_start(out=onesb, in_=onesbd[:, :])
        Rm = const.tile([128, 128], BF16)
        nc.gpsimd.dma_start(out=Rm, in_=rswapd[:, :])
        b4s = const.tile([128, 4], FP32)
        nc.gpsimd.dma_start(out=b4s, in_=b4[:, :])
        kc2s = const.tile([128, S], BF16)
        nc.gpsimd.dma_start(out=kc2s, in_=kc2[:, :])
        ks2s = const.tile([128, S], BF16)
        nc.gpsimd.dma_start(out=ks2s, in_=ks2[:, :])
        qc2s = const.tile([128, S], BF16)
        nc.gpsimd.dma_start(out=qc2s, in_=qc2[:, :])
        qs2s = const.tile([128, S], BF16)
        nc.gpsimd.dma_start(out=qs2s, in_=qs2[:, :])
        epsc = const.tile([128, 1], FP32)
        nc.vector.memset(epsc, EPS)
        # causal band masks generated on-device (keeps them out of the
        # startup HBM burst): mks[p, j, f] = 1 iff f >= p + 128*j
        mks = const.tile([128, 4, SC_ATT], BF16)
        nc.gpsimd.memset(mks, 1.0)
        for j in range(4):
            nc.gpsimd.affine_select(
                out=mks[:, j, :], in_=mks[:, j, :],
                compare_op=mybir.AluOpType.is_ge, fill=0.0,
                base=-128 * j, channel_multiplier=-1,
                pattern=[[1, SC_ATT]])

        rawq = [rawp.tile([128, S], BF16, tag=f"raw{i}", name=f"raw{i}")
                for i in range(3)]
        qhat = [atp.tile([128, S], BF16, tag=f"qh{i}", name=f"qh{i}")
                for i in range(2)]
        khat = atp.tile([128, S], BF16, tag="kh")
        attnT = [atp.tile([128, S], BF16, tag=f"attnT{h}", name=f"attnT{h}")
                 for h in range(2)]
        vsb = vpool.tile([128, 16, HD], BF16, tag="vsb")
        heads = [
            (rawq[2], khat, kc2s, ks2s),
            (rawq[0], qhat[0], qc2s, qs2s),
            (rawq[1], qhat[1], qc2s, qs2s),
        ]
        raw3s = {}

        # ================================================================
        # Software-pipelined emission: stage lag guarantees every
        # instruction's inputs are a full pipeline iteration old, so no
        # engine stream ever blocks at a phase boundary.
        #   iter sc: proj(sc) | rope+V(sc-1) | attention(sc-2) | oproj(sc-3)
        # ================================================================
        for it in range(NCH + 3):
            # ---- stage 1: qkv projection ------------------------------
            if it < NCH:
                sc = it
                sl = bass.ts(sc, SC_ATT)
                htsA = hpool.tile([128, 8, SC_ATT], BF16, tag="htsA")
                nc.sync.dma_start(out=htsA, in_=hT[:, sc, 0:8, :])
                htsB = hpool.tile([128, 8, SC_ATT], BF16, tag="htsB")
                nc.scalar.dma_start(out=htsB, in_=hT[:, sc, 8:16, :])
                if sc == 0:
                    # remaining weights follow htsB on the scalar ring;
                    # woT rides the SWDGE queue (needed latest)
                    for oc in (0, 1, 3):
                        wt = const.tile([128, 16, HD], BF16,
                                        name=f"wts{oc}", tag=f"wts{oc}")
                        nc.scalar.dma_start(out=wt, in_=wts_d[oc][:, :, :])
                        wts[oc] = wt
                    woTs = const.tile([128, 2, HID], BF16)
                    nc.gpsimd.dma_start(out=woTs, in_=woT[:, :, :])
                for oc in (2, 0, 1, 3):
                    ps = psA.tile([128, SC_ATT], FP32, tag="mm")
                    for kt in range(16):
                        src_h = htsA if kt < 8 else htsB
                        nc.tensor.matmul(
                            ps, lhsT=wts[oc][:, kt, :],
                            rhs=src_h[:, kt % 8, :],
                            start=(kt == 0), stop=(kt == 15))
                    if oc == 3:
                        raw3 = wpool.tile([128, SC_ATT], BF16, tag="raw3",
                                          bufs=2)
                        nc.vector.tensor_scalar_add(raw3, ps,
                                                    b4s[:, oc:oc + 1])
                        raw3s[sc] = raw3
                    else:
                        nc.vector.tensor_scalar_add(
                            rawq[oc][:, sl], ps, b4s[:, oc:oc + 1])

            # ---- stage 2: rmsnorm + rope + V tiles --------------------
            if 1 <= it <= NCH:
                sc = it - 1
                sl = bass.ts(sc, SC_ATT)
                for raw, dst, wc2_, ws2_ in heads:
                    sq = wpool.tile([128, SC_ATT], BF16, tag="sq", bufs=2)
                    nc.vector.tensor_mul(sq, raw[:, sl], raw[:, sl])
                    ssum = psN.tile([128, SC_ATT], FP32, tag="norm")
                    nc.tensor.matmul(ssum, lhsT=onesb, rhs=sq,
                                     start=True, stop=True)
                    lnb = wpool.tile([128, SC_ATT], BF16, tag="lnb", bufs=2)
                    nc.scalar.activation(lnb, ssum, AF.Ln,
                                         scale=1.0 / HD, bias=epsc)
                    nc.scalar.activation(lnb, lnb, AF.Exp, scale=-0.5)
                    rtp = psN.tile([128, SC_ATT], FP32, tag="norm")
                    nc.tensor.matmul(rtp, lhsT=Rm, rhs=raw[:, sl],
                                     start=True, stop=True)
                    t1 = wpool.tile([128, SC_ATT], BF16, tag="tt", bufs=3)
                    nc.vector.tensor_mul(t1, raw[:, sl], wc2_[:, sl])
                    t2 = wpool.tile([128, SC_ATT], BF16, tag="tt", bufs=3)
                    nc.vector.tensor_mul(t2, rtp, ws2_[:, sl])
                    t3 = wpool.tile([128, SC_ATT], BF16, tag="tt", bufs=3)
                    nc.vector.tensor_add(t3, t1, t2)
                    nc.vector.tensor_mul(dst[:, sl], t3, lnb)
                raw3 = raw3s.pop(sc)
                for j in range(4):
                    tt = 4 * sc + j
                    vps = psN.tile([128, 128], BF16, tag="norm")
                    nc.tensor.transpose(vps,
                                        raw3[:, bass.ts(j, 128)], ident)
                    nc.vector.tensor_copy(vsb[:, tt, :], vps)

            # ---- stage 3: attention, both heads -----------------------
            # The last query chunk is processed in two 256-wide halves:
            # the first half skips the top key tiles entirely (less exp
            # work) and its o-projection overlaps the second half's
            # attention, shortening the serial tail of the kernel.
            if 2 <= it <= NCH + 1:
                sc = it - 2
                if sc < NCH - 1:
                    subs = [(sc * SC_ATT, SC_ATT)]
                else:
                    subs = [(sc * SC_ATT, SC_ATT // 2),
                            (sc * SC_ATT + SC_ATT // 2, SC_ATT // 2)]
                for q0, qn in subs:
                    for h in range(2):
                        ntt = (q0 + qn) // 128
                        outps = psO.tile([128, qn], FP32, tag="attnout")
                        dacc = dpool.tile([128, qn], F32R, tag="dacc")
                        t0 = 0
                        paired = qn * 2 * 4 <= 2048  # 2 tiles fit one bank
                        while t0 < ntt:
                            gn = min(4, ntt - t0)
                            pg = ppool.tile([128, 4, qn], BF16, tag="pt")
                            if paired:
                                # two key tiles share one PSUM bank and a
                                # single exp ACTIVATE (halves ACT op count)
                                for jp in range(0, gn, 2):
                                    scp2 = psC.tile([128, 2, qn], FP32,
                                                    tag="score")
                                    for u in range(2):
                                        tt = t0 + jp + u
                                        nc.tensor.matmul(
                                            scp2[:, u, :],
                                            lhsT=khat[:, bass.ts(tt, 128)],
                                            rhs=qhat[h][:, bass.ds(q0, qn)],
                                            start=True, stop=True)
                                    nc.scalar.activation(
                                        pg[:, jp:jp + 2, :], scp2,
                                        AF.Exp, scale=SCALE)
                                    for u in range(2):
                                        tt = t0 + jp + u
                                        j = jp + u
                                        jd = tt - q0 // 128
                                        if jd >= 0:
                                            nc.vector.tensor_mul(
                                                pg[:, j, :], pg[:, j, :],
                                                mks[:, jd, 0:qn])
                                        nc.tensor.matmul(
                                            outps, lhsT=vsb[:, tt, :],
                                            rhs=pg[:, j, :],
                                            start=(tt == 0),
                                            stop=(tt == ntt - 1))
                            else:
                                for j in range(gn):
                                    tt = t0 + j
                                    scp = psC.tile([128, qn], FP32,
                                                   tag="score")
                                    nc.tensor.matmul(
                                        scp, lhsT=khat[:, bass.ts(tt, 128)],
                                        rhs=qhat[h][:, bass.ds(q0, qn)],
                                        start=True, stop=True)
                                    nc.scalar.activation(pg[:, j, :], scp,
                                                         AF.Exp,
                                                         scale=SCALE)
                                    jd = tt - q0 // 128
                                    if jd >= 0:  # diagonal: zero t > s
                                        nc.vector.tensor_mul(
                                            pg[:, j, :], pg[:, j, :],
                                            mks[:, jd, 0:qn])
                                    nc.tensor.matmul(outps,
                                                     lhsT=vsb[:, tt, :],
                                                     rhs=pg[:, j, :],
                                                     start=(tt == 0),
                                                     stop=(tt == ntt - 1))
                            if gn == 4:
                                ga = ppool.tile([128, qn], BF16, tag="ga",
                                                bufs=2)
                                nc.vector.tensor_add(ga, pg[:, 0, :],
                                                     pg[:, 1, :])
                                gb = ppool.tile([128, qn], BF16, tag="gb",
                                                bufs=1)
                                nc.vector.tensor_add(gb, pg[:, 2, :],
                                                     pg[:, 3, :])
                                if t0 == 0:
                                    nc.vector.tensor_add(dacc, ga, gb)
                                else:
                                    gc = ppool.tile([128, qn], BF16,
                                                    tag="ga", bufs=2)
                                    nc.vector.tensor_add(gc, ga, gb)
                                    nc.vector.tensor_add(dacc, dacc, gc)
                            else:  # trailing pair (only in split chunks)
                                ga = ppool.tile([128, qn], BF16, tag="ga",
                                                bufs=2)
                                nc.vector.tensor_add(ga, pg[:, 0, :],
                                                     pg[:, 1, :])
                                nc.vector.tensor_add(dacc, dacc, ga)
                            t0 += gn
                        drep = psN.tile([128, qn], FP32, tag="norm")
                        nc.tensor.matmul(drep, lhsT=ones128, rhs=dacc,
                                         start=True, stop=True)
                        drec = wpool.tile([128, qn], FP32, tag="drec",
                                          bufs=2)
                        nc.vector.reciprocal_approx_fast(drec, drep)
                        nc.vector.tensor_mul(attnT[h][:, bass.ds(q0, qn)],
                                             outps, drec)

            # ---- stage 4: o projection --------------------------------
            if it >= 3:
                sc = it - 3
                # late chunks run with no proj/attention stages left, so
                # their oproj can also cycle through the idle psA (and,
                # for the final chunk, psC) banks — deeper PSUM pipeline
                # keeps the PE from stalling on PSUM->SBUF drains.
                if sc == NCH - 1:
                    opsum = [(psB, "omm"), (psA, "mm"), (psC, "score")]
                elif sc == NCH - 2:
                    opsum = [(psB, "omm"), (psA, "mm")]
                else:
                    opsum = [(psB, "omm")]
                pi = 0
                for st in range(4 * sc, 4 * sc + 4):
                    # whole output row block [128, HID] is staged in one
                    # SBUF tile and stored with a single large DMA
                    osb = opool.tile([128, HID], BF16, tag="osb")
                    for jp in range(HID // SC_ATT // 2):  # jc pairs
                        pool_, tag_ = opsum[pi % len(opsum)]
                        pi += 1
                        opsa = pool_.tile([128, SC_ATT], FP32, tag=tag_)
                        opsb = pool_.tile([128, SC_ATT], FP32, tag=tag_)
                        # jc pair shares each head's lhsT (one LDWEIGHTS
                        # per head instead of per matmul)
                        for h in range(2):
                            for ops, jc in ((opsa, 2 * jp),
                                            (opsb, 2 * jp + 1)):
                                nc.tensor.matmul(
                                    ops,
                                    lhsT=attnT[h][:, bass.ts(st, 128)],
                                    rhs=woTs[:, h, bass.ts(jc, SC_ATT)],
                                    start=(h == 0), stop=(h == 1))
                        nc.vector.tensor_copy(
                            osb[:, bass.ts(2 * jp, SC_ATT)], opsa)
                        nc.scalar.copy(
                            osb[:, bass.ts(2 * jp + 1, SC_ATT)], opsb)
                    if sc == NCH - 1 and st == 4 * sc + 3:
                        # very last row block: halves on both rings so the
                        # final store drain is not serialized on one ring
                        nc.sync.dma_start(
                            out=outp[bass.ts(st, 128), 0:HID // 2],
                            in_=osb[:, 0:HID // 2])
                        nc.scalar.dma_start(
                            out=outp[bass.ts(st, 128), HID // 2:HID],
                            in_=osb[:, HID // 2:HID])
                    else:
                        eng = nc.sync if st % 2 == 0 else nc.scalar
                        eng.dma_start(out=outp[bass.ts(st, 128), :],
                                      in_=osb)

    nc.compile()
    return nc


def _prep_inputs(hidden_states, cos, sin, wqkv, bqkv, wo, q_norm_w, k_norm_w):
    """Host-side layout prep + per-core sharding. All device tensors are
    pre-swizzled so every DMA has long contiguous per-partition runs."""
    import ml_dtypes
    bf16 = ml_dtypes.bfloat16
    f32 = np.float32
    hTn = np.ascontiguousarray(hidden_states.reshape(S, HID).T).astype(bf16)
    hTh = np.ascontiguousarray(
        hTn.reshape(16, 128, S // SC_ATT, SC_ATT).transpose(1, 2, 0, 3)
    )  # [p, sc, kt, s] — chunk-major so chunk loads are contiguous
    cosT = cos.T.astype(f32)  # [64, S]
    sinT = sin.T.astype(f32)
    cs2 = np.concatenate([cosT, cosT], axis=0)  # [128, S]
    ss2 = np.concatenate([sinT, sinT], axis=0)
    qwv = q_norm_w.astype(f32).reshape(128, 1)
    kwv = k_norm_w.astype(f32).reshape(128, 1)
    qwsv = np.concatenate([q_norm_w[64:], q_norm_w[:64]]).astype(
        f32).reshape(128, 1)
    kwsv = np.concatenate([k_norm_w[64:], k_norm_w[:64]]).astype(
        f32).reshape(128, 1)
    qc2 = np.ascontiguousarray(qwv * cs2).astype(bf16)
    qs2 = np.ascontiguousarray(qwsv * ss2).astype(bf16)
    kc2 = np.ascontiguousarray(kwv * cs2).astype(bf16)
    ks2 = np.ascontiguousarray(kwsv * ss2).astype(bf16)
    ones_np = np.ones((128, 128), dtype=f32)
    onesb_np = np.ones((128, 128), dtype=bf16)
    rt = np.zeros((128, 128), dtype=f32)
    rt[np.arange(64) + 64, np.arange(64)] = -1.0   # R^T[d+64, d] = -1
    rt[np.arange(64), np.arange(64) + 64] = 1.0    # R^T[d-64, d] = +1
    rt = rt.astype(bf16)

    in_maps = []
    for c in range(N_CORES):
        kvh = c // 2
        rows = list(range(2 * c * HD, (2 * c + 2) * HD))          # q0, q1
        rows += list(range(NH * HD + kvh * HD, NH * HD + (kvh + 1) * HD))  # k
        rows += list(range((NH + NKV) * HD + kvh * HD,
                           (NH + NKV) * HD + (kvh + 1) * HD))      # v
        w_c = wqkv[rows]                       # [512, HID]
        wTc = np.ascontiguousarray(w_c.T).astype(f32)   # [HID, 512]
        wTk = wTc.reshape(16, 128, 512)
        b_c = bqkv[rows].astype(f32)           # [512]
        b4c = np.ascontiguousarray(b_c.reshape(4, 128).T)  # [128, 4]
        woc = wo[:, 2 * c * HD:(2 * c + 2) * HD]  # [HID, 256]
        woTc = np.ascontiguousarray(woc.T).astype(f32)  # [256, HID]
        woTh = np.ascontiguousarray(
            woTc.reshape(2, 128, HID).transpose(1, 0, 2)).astype(bf16)
        im = {
            "hT": hTh, "b4": b4c, "woT": woTh,
            "qc2": qc2, "qs2": qs2, "kc2": kc2, "ks2": ks2,
            "ones": ones_np, "onesb": onesb_np, "rswap": rt,
        }
        for oc in range(4):
            im[f"wT{oc}"] = np.ascontiguousarray(
                wTk[:, :, oc * 128:(oc + 1) * 128].transpose(1, 0, 2)
            ).astype(bf16)
        in_maps.append(im)
    return in_maps


_NC_CACHE = {}


def kernel(hidden_states, cos, sin, k_cache, v_cache, mask,
           wqkv, bqkv, wo, bo, q_norm_w, k_norm_w, kv_write_indices,
           trace=False):
    hidden_states = np.asarray(hidden_states, dtype=np.float32)
    in_maps = _prep_inputs(
        np.asarray(hidden_states), np.asarray(cos), np.asarray(sin),
        np.asarray(wqkv), np.asarray(bqkv), np.asarray(wo),
        np.asarray(q_norm_w), np.asarray(k_norm_w))

    if "nc" not in _NC_CACHE:
        _NC_CACHE["nc"] = build_nc()
    nc = _NC_CACHE["nc"]

    res = run_bass_kernel_spmd(nc, in_maps, core_ids=list(range(N_CORES)),
                               trace=trace)
    out = np.zeros((S, HID), np.float32)
    for rmap in res.results:
        out += np.asarray(rmap["outp"], dtype=np.float32)
    out += np.asarray(bo, dtype=np.float32)[None, :]
    if trace:
        kernel.last_results = res
    return out.reshape(1, S, HID)
